# revision 25
# baseline (speedup 1.0000x reference)
"""Trainium2 Bass kernel for the CACE message-passing GNN (nn_Cace_58291296141968).

Strategy (8 NeuronCores, SPMD), v3:
  - Receivers load-balanced onto 8 cores x 32 subtiles x 16 node slots; edges
    padded to CAP=192 slots/subtile (48 blocks of 128 slots per core).
  - sqrt(multinomial-prefactor) folded into the angular monomials; MP_NORM
    folded into the node table (A rows and V).
  - Node A table stored in FP8-E4M3 (1440 cols) with the bf16 V row embedded
    at byte offset 1440 of each 1536-B row: halves the AllGather and the
    stage-2 dma_gather traffic vs bf16.
  - Stage 2 msg_A uses fp8 DoubleRow matmuls (two radial parities as the two
    k-tiles), halving PE time; lhsT is a parity-masked fp8 S_w (sw2).
  - Persistent bf16 sw1 serves stage-1 seg-sums and stage-2 msg_B; memory
    term folded into the a1 PSUM group via an identity matmul.
"""
import os
import numpy as np
from math import factorial, pi

import ml_dtypes

import concourse.bacc as bacc
import concourse.bass as bass
import concourse.mybir as mybir
import concourse.tile as tile
from concourse.bass_utils import run_bass_kernel_spmd

# ---- problem constants (hardcoded; must match reference.py) ----
ZS = np.array([1, 6, 7, 8], dtype=np.int64)
NZ = 4
NAB = 3
CHAN = 9
MAX_L = 3
N_RBF = 8
N_RB = 8
CUTOFF = 5.5
MP_NORM = 1.0 / 10.0 ** 0.5
N_NODES = 4000
N_EDGES = 48000

def _make_l_list(max_l):
    lst = []
    for l in range(max_l + 1):
        for lx in range(l, -1, -1):
            for ly in range(l - lx, -1, -1):
                lst.append((lx, ly, l - lx - ly))
    return lst

L_LIST = _make_l_list(MAX_L)
N_L = len(L_LIST)                                   # 20
L_OF = np.array([sum(t) for t in L_LIST])
PREF = np.array([factorial(sum(t)) / (factorial(t[0]) * factorial(t[1]) * factorial(t[2]))
                 for t in L_LIST], dtype=np.float64)
L_RANGES = [(0, 1), (1, 4), (4, 10), (10, 20)]
# batched monomial chain: lists of (out_lo, out_hi, par_lo, comp)
_CHAIN_BATCH = [(4, 7, 1, 0), (7, 9, 2, 1), (9, 10, 3, 2),
                (10, 16, 4, 0), (16, 19, 7, 1), (19, 20, 9, 2)]

NC = 8
NSUB = 32
SUBN = 16
CAP = 192                # edge slots per subtile
ES = NSUB * CAP          # 6144 slots/core
EPB = 128
NBLK = ES // EPB         # 48 blocks/core
NROW = NSUB * SUBN       # 512 node rows/core
TABW = 1536              # fp8 table row: 1440 A + 9 bf16 V + pad (bytes % 256 == 0)
GB = 4                   # subtiles per gather call (6 blocks, 768 idx)
P = 128
F32 = mybir.dt.float32
BF16 = mybir.dt.bfloat16
FP8 = mybir.dt.float8e4
I16 = mybir.dt.int16

_PROGRAM = None


def _block_ranges(s):
    """Blocks + partition ranges covering subtile s's 192 slots."""
    g2 = s // 2
    if s % 2 == 0:
        return [(3 * g2, 0, 128), (3 * g2 + 1, 0, 64)]
    return [(3 * g2 + 1, 64, 128), (3 * g2 + 2, 0, 128)]


# ================= host-side sharding prep (index work only) =================
def _prep(positions, shifts, atomic_numbers, edge_index):
    import heapq
    snd = np.asarray(edge_index[0]).astype(np.int64)
    rcv = np.asarray(edge_index[1]).astype(np.int64)
    an = np.asarray(atomic_numbers)
    species = np.searchsorted(ZS, an)
    indeg = np.bincount(rcv, minlength=N_NODES)
    order = np.argsort(-indeg, kind="stable")
    TS = NC * NSUB
    loads = np.zeros(TS, dtype=np.int64)
    counts = np.zeros(TS, dtype=np.int64)
    assign_sub = np.zeros(N_NODES, dtype=np.int64)
    assign_slot = np.zeros(N_NODES, dtype=np.int64)
    heap = [(0, t) for t in range(TS)]
    heapq.heapify(heap)
    for nd in order:
        pending = []
        while True:
            load, t = heapq.heappop(heap)
            if counts[t] < SUBN:
                break
            pending.append((load, t))
        assign_sub[nd] = t
        assign_slot[nd] = counts[t]
        counts[t] += 1
        loads[t] = load + indeg[nd]
        heapq.heappush(heap, (loads[t], t))
        for it in pending:
            heapq.heappush(heap, it)
    assert loads.max() <= CAP, f"subtile edge overflow: {loads.max()} > {CAP}"

    core_of = assign_sub // NSUB
    sub_of = assign_sub % NSUB
    node_row = core_of * NROW + sub_of * SUBN + assign_slot      # node -> global row
    node_of_row = np.full(NC * NROW, -1, dtype=np.int64)
    node_of_row[node_row] = np.arange(N_NODES)
    # table row in tabfull's (slice, core, 256-row) layout (contiguous AG slices)
    tab_row = (sub_of // 16) * (NC * 256) + core_of * 256 + (sub_of % 16) * SUBN + assign_slot

    e_sub = assign_sub[rcv]
    e_order = np.argsort(e_sub, kind="stable")
    bounds = np.searchsorted(e_sub[e_order], np.arange(TS + 1))

    pos = np.asarray(positions, dtype=np.float32)
    shf = np.asarray(shifts, dtype=np.float32)

    geo = np.zeros((NC, 9, ES), dtype=np.float32)                # [comp(SxyzRxyzShxyz), slot]
    geo[:, 3:6, :] = 1.0                                         # benign pad: R=(1,1,1), S=0
    recvoh = np.zeros((NC, SUBN, ES), dtype=np.float32)
    sendrow = np.zeros((NC, ES), dtype=np.int64)
    for t in range(TS):
        c = t // NSUB; s = t % NSUB
        es = e_order[bounds[t]:bounds[t + 1]]
        k = len(es)
        base = s * CAP
        geo[c, 0:3, base:base + k] = pos[snd[es]].T
        geo[c, 3:6, base:base + k] = pos[rcv[es]].T
        geo[c, 6:9, base:base + k] = shf[es].T
        recvoh[c, assign_slot[rcv[es]], base + np.arange(k)] = 1.0
        sendrow[c, base:base + k] = tab_row[snd[es]]

    # device edge-slot layout: slot -> (blk, p) with slot = blk*128 + p
    def to_pb(a):   # [NC, ..., ES] -> [NC, 128, ..., NBLK]
        a2 = a.reshape(a.shape[:-1] + (NBLK, EPB))               # [..., NBLK, 128]
        return np.moveaxis(a2, -1, 1)                            # [NC, 128, ..., NBLK]

    geo_in = np.ascontiguousarray(to_pb(geo).reshape(NC, P, 9 * NBLK))   # [NC, 128, (comp,blk)]
    # recv one-hot in (blk, n) layout: [NC, 128, NBLK, SUBN]
    recv_in = np.ascontiguousarray(
        to_pb(recvoh).transpose(0, 1, 3, 2).reshape(NC, P, NBLK * SUBN))
    # gather idx: per subtile 192 slots; idx k at partition k%16, col sub*12 + k//16
    gidx = np.zeros((NC, P, NSUB * 12), dtype=np.int16)
    for c in range(NC):
        w = sendrow[c].reshape(NSUB, 12, 16).astype(np.int16)    # [sub, k//16, k%16]
        packed = w.transpose(2, 0, 1).reshape(16, NSUB * 12)     # [k%16, (sub, k//16)]
        for g in range(8):
            gidx[c, g * 16:(g + 1) * 16, :] = packed
    # per-edge-slot sender species (pad -> 0) in device layout [NC, 128, NBLK]
    sendsp = np.zeros((NC, ES), dtype=np.int64)
    for t in range(TS):
        c = t // NSUB; s = t % NSUB
        es = e_order[bounds[t]:bounds[t + 1]]
        sendsp[c, s * CAP:s * CAP + len(es)] = species[snd[es]]
    sendsp_in = to_pb(sendsp)                                    # [NC, 128, NBLK]
    # per-node-row species (empty rows -> 0; all their uses are masked/zero)
    rowsp = np.zeros((NC, NROW), dtype=np.int64)
    msk = node_of_row >= 0
    rowsp.reshape(-1)[msk] = species[node_of_row[msk]]
    return dict(geo=geo_in, recv=recv_in, gidx=gidx, sendsp=sendsp_in, rowsp=rowsp,
                node_of_row=node_of_row, node_row=node_row)


def _consts():
    blkdiag = ((np.arange(P)[:, None] % 16) == (np.arange(P)[None, :] % 16)).astype(np.float32)
    nrow = np.tile((np.arange(1, N_RBF + 1) * pi / CUTOFF).astype(np.float32)[None, :], (P, 1))
    sprow = np.tile(np.sqrt(PREF).astype(np.float32)[None, :], (P, 1))   # [128, 20]
    return np.concatenate([blkdiag, nrow, sprow], axis=1)        # [128, 156]


def _parc16():
    parc = np.zeros((P, 16), dtype=np.float32)                   # [par, r] keep r where r%2==par
    for par in range(2):
        for r in range(8):
            if r % 2 == par:
                parc[:, par * 8 + r] = 1.0
    return parc


# ================= device program =================
def _build(sim_mode=False):
    PH = int(os.environ.get("KPHASES", "63"))  # bit0 base,1 s1,2 node1,3 repack,4 s2,5 node2
    nc = bacc.Bacc("TRN2", target_bir_lowering=False, debug=False,
                   num_devices=(1 if sim_mode else NC),
                   dynamic_dma_scratch_size=32768)
    AF = mybir.ActivationFunctionType
    OP = mybir.AluOpType
    DR = mybir.MatmulPerfMode.DoubleRow

    # x_main packs [geo 432 | cons 156 | wpack 308]
    NMAIN = 9 * NBLK + 156 + 308
    NB16 = NBLK * SUBN + NBLK * NAB + 32     # recv16 768 | embsE16 144 | parc16 16 | zeros 16
    x_main = nc.dram_tensor("x_main", [P, NMAIN], F32, kind="ExternalInput")
    x_b16 = nc.dram_tensor("x_b16", [P, NB16], BF16, kind="ExternalInput")
    x_zero = nc.dram_tensor("x_zero", [P, NBLK * 128], BF16, kind="ExternalInput")
    x_gidx = nc.dram_tensor("x_gidx", [P, NSUB * 12], I16, kind="ExternalInput")
    o_b0 = nc.dram_tensor("o_b0", [P, NSUB * 45], F32, kind="ExternalOutput")
    o_b1 = nc.dram_tensor("o_b1", [P, NSUB * 45], F32, kind="ExternalOutput")

    with tile.TileContext(nc) as tc:
        with (
            tc.tile_pool(name="persist", bufs=1) as pp,
            tc.tile_pool(name="work", bufs=2) as wp,
            tc.tile_pool(name="dram", bufs=1, space="DRAM") as dr,
        ):
            # ---------- loads (geo first so the edge phase starts early) ----------
            main = pp.tile([P, NMAIN], F32)
            C1 = 9 * NBLK
            nc.sync.dma_start(main[:, 0:C1], x_main[:, 0:C1])
            nc.sync.dma_start(main[:, C1:NMAIN], x_main[:, C1:NMAIN])
            b16 = pp.tile([P, NB16], BF16)
            nc.sync.dma_start(b16[:], x_b16[:])
            gidx = pp.tile([P, NSUB * 12], I16)
            nc.sync.dma_start(gidx[:], x_gidx[:])
            o = 0
            geo = main[:, o:o + 9 * NBLK]; o += 9 * NBLK
            cons = main[:, o:o + 156]; o += 156
            wpack = main[:, o:o + 308]; o += 308
            blkdiag = cons[:, 0:128]
            nrow = cons[:, 128:136]
            sprow = cons[:, 136:156]
            EM = wpack[:, 212:308]          # pure emb products
            recv16 = b16[:, 0:NBLK * SUBN]
            embsE16 = b16[:, NBLK * SUBN:NBLK * SUBN + NBLK * NAB]
            parc16 = b16[:, NBLK * SUBN + NBLK * NAB:NBLK * SUBN + NBLK * NAB + 16]
            zeros16 = b16[:, NBLK * SUBN + NBLK * NAB + 16:]

            # ---------- one-time derived weights ----------
            rtl16 = []
            for l in range(MAX_L + 1):
                rtl_t = pp.tile([P, P], BF16, tag=f"rtl{l}")
                rtl16.append(rtl_t)
                nc.vector.tensor_tensor(
                    out=rtl_t[:].rearrange("p (s n) -> p s n", s=8),
                    in0=wpack[:, l * 8:(l + 1) * 8][:, :, None].to_broadcast([P, 8, 16]),
                    in1=blkdiag.rearrange("p (s n) -> p s n", s=8),
                    op=OP.mult)
            WT16 = pp.tile([P, 180], BF16)
            nc.scalar.copy(WT16[:], wpack[:, 32:212])
            ident16 = pp.tile([P, P], BF16)
            nc.scalar.copy(ident16[:], blkdiag[:])

            # ---------- per-edge base phase ----------
            D = pp.tile([P, 3 * NBLK], F32)
            nc.vector.tensor_tensor(out=D[:], in0=geo[:, 3 * NBLK:6 * NBLK],
                                    in1=geo[:, 0:3 * NBLK], op=OP.subtract)
            nc.vector.tensor_tensor(out=D[:], in0=D[:], in1=geo[:, 6 * NBLK:9 * NBLK], op=OP.add)
            sq = wp.tile([P, 3 * NBLK], F32, tag="sq")
            nc.vector.tensor_tensor(out=sq[:], in0=D[:], in1=D[:], op=OP.mult)
            r2 = wp.tile([P, NBLK], F32, tag="r2")
            nc.vector.tensor_tensor(out=r2[:], in0=sq[:, 0:NBLK], in1=sq[:, NBLK:2 * NBLK], op=OP.add)
            nc.vector.tensor_tensor(out=r2[:], in0=r2[:], in1=sq[:, 2 * NBLK:3 * NBLK], op=OP.add)
            rr = wp.tile([P, NBLK], F32, tag="rr")
            nc.scalar.activation(rr[:], r2[:], AF.Sqrt)
            rinv = pp.tile([P, NBLK], F32)
            nc.vector.reciprocal(rinv[:], rr[:])
            uu = wp.tile([P, NBLK], F32, tag="uu")
            nc.vector.tensor_scalar_mul(uu[:], rr[:], 1.0 / CUTOFF)
            U = pp.tile([P, 3 * NBLK], F32)
            nc.vector.tensor_tensor(
                out=U[:].rearrange("p (c b) -> p c b", c=3),
                in0=D[:].rearrange("p (c b) -> p c b", c=3),
                in1=rinv[:, None, :].to_broadcast([P, 3, NBLK]), op=OP.mult)
            # bessel args [128, (blk, r)] + range reduction to [-pi, pi): the
            # reduction runs on gpsimd, in parallel with the DVE angular chain
            arg = wp.tile([P, NBLK * 8], F32, tag="arg")
            nc.vector.tensor_tensor(
                out=arg[:].rearrange("p (b r) -> p b r", r=8),
                in0=rr[:, :, None].to_broadcast([P, NBLK, 8]),
                in1=nrow[:, None, :].to_broadcast([P, NBLK, 8]), op=OP.mult)
            # parallel range reduction: k*2pi with k from 3 independent
            # comparisons (DVE+gpsimd), then one extra fold to [-pi, pi)
            ge1 = wp.tile([P, NBLK * 8], F32, tag="ge1")
            ge2 = wp.tile([P, NBLK * 8], F32, tag="ge2")
            ge3 = wp.tile([P, NBLK * 8], F32, tag="ge3")
            nc.vector.tensor_scalar(out=ge1[:], in0=arg[:], scalar1=float(2 * pi),
                                    scalar2=float(2 * pi), op0=OP.is_ge, op1=OP.mult)
            nc.gpsimd.tensor_scalar(out=ge2[:], in0=arg[:], scalar1=float(4 * pi),
                                    scalar2=float(2 * pi), op0=OP.is_ge, op1=OP.mult)
            nc.vector.tensor_scalar(out=ge3[:], in0=arg[:], scalar1=float(6 * pi),
                                    scalar2=float(2 * pi), op0=OP.is_ge, op1=OP.mult)
            nc.gpsimd.tensor_tensor(out=ge2[:], in0=ge2[:], in1=ge3[:], op=OP.add)
            nc.vector.tensor_tensor(out=arg[:], in0=arg[:], in1=ge1[:], op=OP.subtract)
            nc.vector.tensor_tensor(out=arg[:], in0=arg[:], in1=ge2[:], op=OP.subtract)
            nc.vector.tensor_scalar(out=ge1[:], in0=arg[:], scalar1=float(pi),
                                    scalar2=float(2 * pi), op0=OP.is_ge, op1=OP.mult)
            nc.vector.tensor_tensor(out=arg[:], in0=arg[:], in1=ge1[:], op=OP.subtract)
            # angular monomials ang [128, (blk, i)] scaled by sqrt(PREF), on DVE
            # while gpsimd reduces the bessel arguments
            ang = pp.tile([P, NBLK * N_L], F32)
            angv = ang[:].rearrange("p (b i) -> p b i", i=N_L)
            nc.vector.tensor_scalar(out=angv[:, :, 0], in0=uu[:], scalar1=0.0, scalar2=1.0,
                                    op0=OP.mult, op1=OP.add)
            nc.vector.tensor_copy(
                angv[:, :, 1:4],
                U[:].rearrange("p (c b) -> p b c", c=3))
            for lo, hi, plo, c in _CHAIN_BATCH:
                cnt = hi - lo
                nc.vector.tensor_tensor(
                    out=angv[:, :, lo:hi],
                    in0=angv[:, :, plo:plo + cnt],
                    in1=U[:, c * NBLK:(c + 1) * NBLK][:, :, None].to_broadcast([P, NBLK, cnt]),
                    op=OP.mult)
            # fold the sqrt(PREF) prefactor into the bf16 conversion
            ang16 = pp.tile([P, NBLK * N_L], BF16)
            ang16v = ang16[:].rearrange("p (b i) -> p b i", i=N_L)
            nc.vector.tensor_tensor(
                out=ang16v,
                in0=angv[:],
                in1=sprow[:, None, :].to_broadcast([P, NBLK, N_L]),
                op=OP.mult)

            sinv = wp.tile([P, NBLK * 8], F32, tag="sinv")
            nc.scalar.activation(sinv[:], arg[:], AF.Sin)
            # cutoff polynomial
            u2 = wp.tile([P, NBLK], F32, tag="u2")
            nc.vector.tensor_tensor(out=u2[:], in0=uu[:], in1=uu[:], op=OP.mult)
            a1 = wp.tile([P, NBLK], F32, tag="a1")
            nc.vector.tensor_scalar(out=a1[:], in0=uu[:], scalar1=-48.0, scalar2=28.0,
                                    op0=OP.mult, op1=OP.add)
            g21 = wp.tile([P, NBLK], F32, tag="g21")
            nc.vector.tensor_scalar_mul(g21[:], u2[:], 21.0)
            nc.vector.tensor_tensor(out=g21[:], in0=g21[:], in1=a1[:], op=OP.add)
            u6 = wp.tile([P, NBLK], F32, tag="u6")
            nc.vector.tensor_tensor(out=u6[:], in0=u2[:], in1=u2[:], op=OP.mult)
            nc.vector.tensor_tensor(out=u6[:], in0=u6[:], in1=u2[:], op=OP.mult)
            fc = wp.tile([P, NBLK], F32, tag="fc")
            nc.vector.tensor_tensor(out=fc[:], in0=u6[:], in1=g21[:], op=OP.mult)
            nc.vector.tensor_scalar(out=fc[:], in0=fc[:], scalar1=-1.0, scalar2=1.0,
                                    op0=OP.mult, op1=OP.add)
            lt = wp.tile([P, NBLK], F32, tag="lt")
            nc.vector.tensor_scalar(out=lt[:], in0=uu[:], scalar1=1.0, scalar2=None, op0=OP.is_lt)
            nc.vector.tensor_tensor(out=fc[:], in0=fc[:], in1=lt[:], op=OP.mult)
            scal = wp.tile([P, NBLK], F32, tag="scal")
            nc.vector.tensor_tensor(out=scal[:], in0=rinv[:], in1=fc[:], op=OP.mult)
            nc.vector.tensor_scalar_mul(scal[:], scal[:], float(np.sqrt(2.0 / CUTOFF)))
            # rc in bf16 (single rounding from the f32 product)
            rc16 = pp.tile([P, NBLK * 8], BF16)
            nc.vector.tensor_tensor(
                out=rc16[:].rearrange("p (b r) -> p b r", r=8),
                in0=sinv[:].rearrange("p (b r) -> p b r", r=8),
                in1=scal[:, :, None].to_broadcast([P, NBLK, 8]), op=OP.mult)

            # G1 [128, (blk, i, a)] bf16, per 12-block quarter (all-bf16 2x)
            G1 = pp.tile([P, NBLK * N_L * NAB], BF16)

            def build_g1(g8):
                bs = slice(g8 * 12, g8 * 12 + 12)
                nc.vector.tensor_tensor(
                    out=G1[:].rearrange("p (b i a) -> p b i a", i=N_L, a=NAB)[:, bs],
                    in0=ang16v[:, bs, :, None].to_broadcast([P, 12, N_L, NAB]),
                    in1=embsE16.rearrange("p (b a) -> p b a", a=NAB)[:, bs, None, :].to_broadcast([P, 12, N_L, NAB]),
                    op=OP.mult)

            # sw1 [128, (blk, r, n)] bf16: rc x recv one-hot, per 12-block quarter
            sw1 = pp.tile([P, NBLK * P], BF16)

            def build_sw1(g8):
                bs = slice(g8 * 12, g8 * 12 + 12)
                nc.vector.tensor_tensor(
                    out=sw1[:].rearrange("p (b r n) -> p b r n", r=8, n=16)[:, bs],
                    in0=recv16.rearrange("p (b n) -> p b n", n=SUBN)[:, bs, None, :].to_broadcast([P, 12, 8, 16]),
                    in1=rc16[:].rearrange("p (b r) -> p b r", r=8)[:, bs, :, None].to_broadcast([P, 12, 8, 16]),
                    op=OP.mult)

            # parity-split S_w in fp8 for stage-2 DoubleRow msg_A. The zero
            # half is DMA-broadcast-filled (off-engine); only the 6144
            # nonzeros (at r = 2*rp + q, a linear-stride AP) are computed.
            sw2 = pp.tile([P, NBLK * 256], FP8)
            nc.sync.dma_start(out=sw2[:].bitcast(BF16), in_=x_zero[:])
            sw2nz = sw2[:].rearrange("p (b q rp par n) -> p b q rp par n",
                                     q=2, rp=4, par=2, n=16)
            rc16r = rc16[:].rearrange("p (b rp par) -> p b rp par", rp=4, par=2)

            def build_sw2(q, half, eng):
                bs = slice(half * 24, half * 24 + 24)
                eng.tensor_tensor(
                    out=sw2nz[:, bs, q, :, q, :],
                    in0=recv16.rearrange("p (b n) -> p b n", n=SUBN)[:, bs, None, :].to_broadcast([P, 24, 4, 16]),
                    in1=rc16r[:, bs, :, q][:, :, :, None].to_broadcast([P, 24, 4, 16]),
                    op=OP.mult)

            def symmetrize_pool(bv, sv, ns):
                # bv [P,ns,5,c]; sv [P,ns,20,c]: sum-of-squares tree on gpsimd
                # (bv[:,:,0,:] filled by the caller via Act copy)
                s5 = wp.tile([P, 8 * 5 * CHAN], F32, tag="ps5")
                v5 = s5[:].rearrange("p (s i c) -> p s i c", i=5, c=CHAN)[:, 0:ns]
                s3 = wp.tile([P, 8 * 3 * CHAN], F32, tag="ps3")
                v3 = s3[:].rearrange("p (s i c) -> p s i c", i=3, c=CHAN)[:, 0:ns]
                TT = nc.gpsimd.tensor_tensor
                nc.gpsimd.tensor_copy(bv[:, :, 1, :], sv[:, :, 0, :])
                # l=1: i 1..4
                TT(out=v3[:, :, 0, :], in0=sv[:, :, 1, :], in1=sv[:, :, 2, :], op=OP.add)
                TT(out=bv[:, :, 2, :], in0=v3[:, :, 0, :], in1=sv[:, :, 3, :], op=OP.add)
                # l=2: i 4..10
                TT(out=v3[:], in0=sv[:, :, 4:7, :], in1=sv[:, :, 7:10, :], op=OP.add)
                TT(out=v5[:, :, 0, :], in0=v3[:, :, 0, :], in1=v3[:, :, 1, :], op=OP.add)
                TT(out=bv[:, :, 3, :], in0=v5[:, :, 0, :], in1=v3[:, :, 2, :], op=OP.add)
                # l=3: i 10..20
                TT(out=v5[:], in0=sv[:, :, 10:15, :], in1=sv[:, :, 15:20, :], op=OP.add)
                TT(out=v3[:, :, 0:2, :], in0=v5[:, :, 0:2, :], in1=v5[:, :, 2:4, :], op=OP.add)
                TT(out=v3[:, :, 2, :], in0=v3[:, :, 0, :], in1=v3[:, :, 1, :], op=OP.add)
                TT(out=bv[:, :, 4, :], in0=v3[:, :, 2, :], in1=v5[:, :, 4, :], op=OP.add)

            A16 = pp.tile([P, NSUB * 180], BF16)
            A8 = pp.tile([P, NSUB * 180], FP8)
            mem16 = pp.tile([P, NSUB * 180], BF16)
            B0_all = pp.tile([P, NSUB * 45], F32)
            B1_all = pp.tile([P, NSUB * 45], F32)
            red1 = pp.tile([P, NSUB * CHAN], F32)
            chic = pp.tile([16, NSUB * CHAN], F32)
            Vsb = pp.tile([16, NSUB * CHAN], BF16)

            tabsh = dr.tile([NROW, TABW], FP8)
            tabfull = dr.tile([NC * NROW, TABW], FP8)

            # ---------- stage 1: seg-sum + RT for all 4 groups first (PE/Act
            # critical path unblocked), node-level phases stream behind ----------
            s1ctx = tc.tile_pool(name="ps_s1", bufs=2, space="PSUM")
            ps_s1 = s1ctx.__enter__()
            t1gs = []
            if PH & 2:
                build_g1(0)
                build_sw1(0)
            for g8 in range(4 if (PH & 2) else 0):
                if g8 < 3:
                    build_g1(g8 + 1)
                    build_sw1(g8 + 1)
                t1g = ps_s1.tile([P, 480], F32, space="PSUM", tag="t1g", bufs=4)
                t1gs.append(t1g)
                pend = []
                for j in range(9):
                    if j < 8:
                        s = g8 * 8 + j
                        t0 = ps_s1.tile([P, 60], F32, space="PSUM", tag="t0", bufs=3)
                        ranges = _block_ranges(s)
                        for mi, (blk, p0, p1) in enumerate(ranges):
                            nc.tensor.matmul(t0[:], lhsT=sw1[p0:p1, blk * 128:(blk + 1) * 128],
                                             rhs=G1[p0:p1, blk * 60:(blk + 1) * 60],
                                             start=(mi == 0), stop=(mi == len(ranges) - 1))
                        t0c = wp.tile([P, 60], BF16, tag="t0c", bufs=4)
                        nc.scalar.copy(t0c[:], t0[:])
                        pend.append((j, t0c))
                    if (j > 0 or g8 > 0) and pend and (j == 8 or len(pend) > 1):
                        jj, t0cp = pend.pop(0)
                        for l, (a, b) in enumerate(L_RANGES):
                            nc.tensor.matmul(
                                t1g[:, jj * 60 + a * NAB: jj * 60 + b * NAB],
                                lhsT=rtl16[l][:], rhs=t0cp[:, a * NAB:b * NAB],
                                start=True, stop=True)
            for g8 in range(4 if (PH & 4) else 0):
                # ---- group node-level: A16, A8, B0, chi, V, repack, AG slice ----
                t1g = t1gs[g8]
                sl = slice(g8 * 1440, (g8 + 1) * 1440)
                sl45 = slice(g8 * 360, (g8 + 1) * 360)
                sl9 = slice(g8 * 72, (g8 + 1) * 72)
                # A16 holds the unscaled A (bf16); MP_NORM enters only in the
                # fp8 A8 copy (table) and chic
                nc.vector.tensor_tensor(
                    out=A16[:, sl].rearrange("p (j ia b) -> p j ia b", j=8, b=NAB),
                    in0=t1g[:].rearrange("p (j ia) -> p j ia", j=8)[:, :, :, None].to_broadcast([P, 8, 60, NAB]),
                    in1=EM[:, g8 * 24:(g8 + 1) * 24].rearrange("p (j b) -> p j b", b=NAB)[:, :, None, :].to_broadcast([P, 8, 60, NAB]),
                    op=OP.mult)
                nc.scalar.activation(A8[:, sl], A16[:, sl], AF.Copy, scale=float(MP_NORM))
                scr = wp.tile([P, 1440], F32, tag="scr")
                nc.scalar.activation(scr[:], A16[:, sl], AF.Square)
                bv = B0_all[:, sl45].rearrange("p (s l c) -> p s l c", l=5, c=CHAN)
                sv = scr[:].rearrange("p (s i c) -> p s i c", i=N_L, c=CHAN)
                nc.gpsimd.tensor_copy(
                    bv[:, :, 0, :],
                    A16[:, sl].rearrange("p (s i c) -> p s i c", i=N_L, c=CHAN)[:, :, 0, :])
                rv = red1[:, sl9].rearrange("p (s c) -> p s c", c=CHAN)
                if g8 == 3:
                    # last group: chi computed straight from the squares so the
                    # final AG slice doesn't wait on the symmetrize tree
                    nc.vector.tensor_reduce(
                        out=rv, in_=sv.transpose([0, 1, 3, 2]),
                        axis=mybir.AxisListType.X, op=OP.add)
                    nc.vector.tensor_tensor(out=rv, in0=rv, in1=bv[:, :, 0, :], op=OP.add)
                    symmetrize_pool(bv, sv, 8)
                else:
                    symmetrize_pool(bv, sv, 8)
                    nc.vector.tensor_reduce(
                        out=rv, in_=bv.transpose([0, 1, 3, 2]),
                        axis=mybir.AxisListType.X, op=OP.add)
                chips = ps_s1.tile([16, 72], F32, space="PSUM", tag="chips", bufs=1)
                nc.tensor.matmul(chips[:], lhsT=blkdiag[:, 0:16], rhs=red1[:, sl9],
                                 start=True, stop=True)
                nc.vector.tensor_scalar_mul(chic[:, sl9], chips[:], float(MP_NORM))
                nc.vector.tensor_tensor(
                    out=Vsb[:, sl9].rearrange("p (s a b) -> p s a b", a=NAB, b=NAB),
                    in0=chic[:, sl9].rearrange("p (s a b) -> p s a b", a=NAB, b=NAB),
                    in1=EM[0:16, g8 * 24:(g8 + 1) * 24].rearrange("p (s a) -> p s a", a=NAB)[:, :, :, None].to_broadcast([16, 8, NAB, NAB]),
                    op=OP.mult)
                if (PH & 8) and g8 % 2 == 1:
                    # repack super-group: A rows (fp8) + V column (bf16 in the
                    # row pad) for 16 subtiles (256 table rows); then AG slice
                    sg = g8 // 2
                    ssl = slice(sg * 2880, (sg + 1) * 2880)
                    ssl9 = slice(sg * 144, (sg + 1) * 144)
                    for sp in range(8):
                        nc.sync.dma_start(
                            out=tabsh[:].rearrange("(s n) w -> n s w", n=SUBN)[:, sg * 16:(sg + 1) * 16, sp * 180:(sp + 1) * 180],
                            in_=A8[sp * 16:(sp + 1) * 16, ssl].rearrange("n (s f) -> n s f", f=180))
                    nc.sync.dma_start(
                        out=tabsh[:].rearrange("(s n) w -> n s w", n=SUBN)[:, sg * 16:(sg + 1) * 16, 1440:1458].bitcast(BF16),
                        in_=Vsb[:, ssl9].rearrange("n (s c) -> n s c", c=CHAN))
                    rsl = slice(sg * 256, (sg + 1) * 256)
                    if sim_mode:
                        # stand-in for the sliced AllGather: 4 local copies per
                        # slice model the 8-core AG of the 0.77MB/rank fp8
                        # shard (same total bytes as the real collective)
                        for _cc in range(4):
                            nc.sync.dma_start(
                                tabfull[sg * NC * 256 + _cc * 256:
                                        sg * NC * 256 + (_cc + 1) * 256, :],
                                tabsh[rsl, :])
                    else:
                        # tabfull rows are (slice, core, 256): slice output is
                        # the contiguous rank-major block for this slice
                        nc.gpsimd.collective_compute(
                            "AllGather", mybir.AluOpType.bypass,
                            replica_groups=[list(range(NC))],
                            ins=[tabsh[rsl, :]],
                            outs=[tabfull[sg * NC * 256:(sg + 1) * NC * 256, :]])
            s1ctx.__exit__(None, None, None)

            # sw2 fp8 build: fills the AllGather window
            if PH & 16:
                build_sw2(0, 0, nc.vector)
                build_sw2(1, 0, nc.gpsimd)
                build_sw2(0, 1, nc.vector)
                build_sw2(1, 1, nc.vector)
            sw2v = sw2[:].rearrange("p (b q rn) -> p b q rn", q=2, rn=128)
            # memory term (bf16 fast path; WT pre-divided by MP_NORM on host);
            # emitted late so it lands in the AllGather/stage-2 DVE idle time
            nc.vector.tensor_tensor(
                out=mem16[:].rearrange("p (s f) -> p s f", f=180),
                in0=A16[:].rearrange("p (s f) -> p s f", f=180),
                in1=WT16[:, None, :].to_broadcast([P, NSUB, 180]),
                op=OP.mult)

            # ---------- stage 2 (1-deep software pipeline: pair gg's
            # gather-independent matmuls are emitted before pair gg-1's
            # t2s-dependent tail, so PE never stalls on the DVE hop) ----------
            s2ctx = tc.tile_pool(name="ps_s2", bufs=2, space="PSUM")
            ps_s2 = s2ctx.__enter__()
            gat4 = None
            NP = NSUB // 2 if (PH & 16) else 0
            hist = {}
            for gg in range(NP + 1 if NP else 0):
                if gg < NP:
                    if gg % 2 == 0:
                        g4 = gg // 2
                        gat4 = wp.tile([P, 6, TABW], FP8, tag="gat", bufs=3)
                        nc.gpsimd.dma_gather(gat4[:], tabfull[:],
                                             gidx[:, g4 * 48:(g4 + 1) * 48],
                                             GB * CAP, GB * CAP, TABW)
                    b3 = (gg % 2) * 3             # this pair's blocks within gat4
                    gatv = gat4[:, b3:b3 + 3, 1440:1458].bitcast(BF16)
                    G2 = wp.tile([P, 3, 180], BF16, tag="g2", bufs=3)
                    nc.vector.tensor_tensor(
                        out=G2[:].rearrange("p b (i c) -> p b i c", c=CHAN),
                        in0=ang16v[:, 3 * gg:3 * gg + 3, :, None].to_broadcast([P, 3, N_L, CHAN]),
                        in1=gatv[:, :, None, :].to_broadcast([P, 3, N_L, CHAN]),
                        op=OP.mult)
                    t2pair = ps_s2.tile([P, 360], F32, space="PSUM", tag="t2", bufs=3)
                    a1pair = ps_s2.tile([P, 360], F32, space="PSUM", tag="a1p", bufs=3)
                    hist[gg] = (t2pair, a1pair)
                    for s2 in range(2):
                        s = gg * 2 + s2
                        osl = slice(s2 * 180, (s2 + 1) * 180)
                        ranges = _block_ranges(s)
                        for mi, (blk, p0, p1) in enumerate(ranges):
                            bloc = blk - 3 * gg + b3
                            nc.tensor.matmul(
                                t2pair[:, osl],
                                lhsT=sw1[p0:p1, blk * 128:(blk + 1) * 128],
                                rhs=G2[p0:p1, bloc - b3, :],
                                start=(mi == 0), stop=(mi == len(ranges) - 1))
                        # msg_A: fp8 DoubleRow, radial parity pair as the k-tiles
                        for ri, (blk, p0, p1) in enumerate(ranges):
                            bloc = blk - 3 * gg + b3
                            for k in range(4):
                                nc.tensor.matmul(
                                    a1pair[k * 32:(k + 1) * 32, osl],
                                    lhsT=sw2v[p0:p1, blk, :, k * 32:(k + 1) * 32],
                                    rhs=gat4[p0:p1, bloc, 2 * k * 180:(2 * k + 2) * 180].rearrange(
                                        "p (q f) -> p q f", q=2),
                                    start=(ri == 0), stop=False,
                                    perf_mode=DR, tile_position=(p0, k * 32))
                if gg == 0:
                    continue
                gp = gg - 1
                t2pair, a1pair = hist.pop(gp)
                t2s = wp.tile([P, 360], BF16, tag="t2s", bufs=3)
                for s2 in range(2):
                    s = gp * 2 + s2
                    osl = slice(s2 * 180, (s2 + 1) * 180)
                    nc.vector.tensor_tensor(
                        out=t2s[:, osl].rearrange("p (i a b) -> p i a b", a=NAB, b=NAB),
                        in0=t2pair[:, osl].rearrange("p (i a b) -> p i a b", a=NAB, b=NAB),
                        in1=EM[:, s * NAB:(s + 1) * NAB][:, None, None, :].to_broadcast([P, N_L, NAB, NAB]),
                        op=OP.mult)
                    for l, (a, b) in enumerate(L_RANGES):
                        nc.tensor.matmul(
                            a1pair[:, s2 * 180 + a * CHAN: s2 * 180 + b * CHAN],
                            lhsT=rtl16[l][:], rhs=t2s[:, s2 * 180 + a * CHAN: s2 * 180 + b * CHAN],
                            start=False, stop=False)
                    # fold the memory term into the PSUM group (identity matmul)
                    nc.tensor.matmul(
                        a1pair[:, osl], lhsT=ident16[:],
                        rhs=mem16[:, s * 180:(s + 1) * 180],
                        start=False, stop=True)
                if not (PH & 32):
                    continue
                # ---- stage 2 node-level per pair: B1 + output ----
                sl45 = slice(gp * 90, (gp + 1) * 90)
                bv = B1_all[:, sl45].rearrange("p (s l c) -> p s l c", l=5, c=CHAN)
                a1v = a1pair[:].rearrange("p (s i c) -> p s i c", i=N_L, c=CHAN)
                scr1 = wp.tile([P, 360], F32, tag="scr1", bufs=3)
                nc.scalar.activation(scr1[:], a1pair[:], AF.Square)
                nc.scalar.copy(bv[:, :, 0, :], a1v[:, :, 0, :])
                sv = scr1[:].rearrange("p (s i c) -> p s i c", i=N_L, c=CHAN)
                if gp % 2 == 0:
                    for l, (a, b) in enumerate(L_RANGES):
                        nc.vector.tensor_reduce(
                            out=bv[:, :, l + 1, :],
                            in_=sv[:, :, a:b, :].transpose([0, 1, 3, 2]),
                            axis=mybir.AxisListType.X, op=OP.add)
                else:
                    symmetrize_pool(bv, sv, 2)
                nc.sync.dma_start(o_b1[:, sl45], B1_all[:, sl45])
            s2ctx.__exit__(None, None, None)
            nc.sync.dma_start(o_b0[:], B0_all[:])
            if not (PH & 4):
                nc.sync.dma_start(o_b0[:, 0:156], cons[:])
            if not (PH & 32):
                nc.sync.dma_start(o_b1[:, 0:156], cons[:])
    nc.compile()
    return nc


# ================= public entry =================
def kernel(positions, shifts, W_emb, W_rt, W_nm, atomic_numbers, edge_index):
    global _PROGRAM
    prep = _prep(positions, shifts, atomic_numbers, edge_index)
    consts = _consts()
    if _PROGRAM is None:
        _PROGRAM = _build()
    nc = _PROGRAM
    wemb = np.asarray(W_emb, dtype=np.float32)
    wrt = np.asarray(W_rt, dtype=np.float32)
    wnm = np.asarray(W_nm, dtype=np.float32)
    # host-replicated weight patterns (pure tiling/gathers of the small weights)
    pg = np.arange(P) // 16                                   # r|s' group per partition
    rtlw = wrt[:, pg, :].transpose(1, 0, 2).reshape(P, 32)    # [p, (l, s')] = W_rt[l, p//16, s']
    wtp = wnm[0, pg][:, L_OF, :].reshape(P, 180)
    parc = _parc16()
    in_maps = []
    for c in range(NC):
        em = wemb[prep["rowsp"][c].reshape(NSUB, SUBN)]       # [sub, n, a]
        em = em[:, np.arange(P) % 16, :].transpose(1, 0, 2).reshape(P, NSUB * NAB)
        wpack = np.concatenate([rtlw, wtp, em], axis=1).astype(np.float32)
        embse = wemb[prep["sendsp"][c]].reshape(P, NBLK * NAB).astype(np.float32)
        main = np.ascontiguousarray(np.concatenate(
            [prep["geo"][c], consts, wpack],
            axis=1).astype(np.float32))
        b16 = np.ascontiguousarray(np.concatenate(
            [prep["recv"][c], embse, parc, np.zeros((P, 16), np.float32)],
            axis=1).astype(ml_dtypes.bfloat16))
        in_maps.append(dict(x_main=main, x_b16=b16, x_gidx=prep["gidx"][c],
                            x_zero=np.zeros((P, NBLK * 128), ml_dtypes.bfloat16)))
    res = run_bass_kernel_spmd(nc, in_maps, list(range(NC))).results
    # unshard: [128=(s',n), (sub, l, c)] -> node rows
    out = np.zeros((N_NODES, N_RB, 5, CHAN, 2), dtype=np.float32)
    node_of_row = prep["node_of_row"]
    for c in range(NC):
        for mp, name in ((0, "o_b0"), (1, "o_b1")):
            arr = np.asarray(res[c][name], dtype=np.float32).reshape(8, SUBN, NSUB, 5, CHAN)
            rows = arr.transpose(2, 1, 0, 3, 4).reshape(NROW, N_RB, 5, CHAN)
            valid = node_of_row[c * NROW:(c + 1) * NROW] >= 0
            out[node_of_row[c * NROW:(c + 1) * NROW][valid], :, :, :, mp] = rows[valid]
    return out


# revision 26
# speedup vs baseline: 1.0538x; 1.0538x over previous
"""Trainium2 Bass kernel for the CACE message-passing GNN (nn_Cace_58291296141968).

Strategy (8 NeuronCores, SPMD), v3:
  - Receivers load-balanced onto 8 cores x 32 subtiles x 16 node slots; edges
    padded to CAP=192 slots/subtile (48 blocks of 128 slots per core).
  - sqrt(multinomial-prefactor) folded into the angular monomials; MP_NORM
    folded into the node table (A rows and V).
  - Node A table stored in FP8-E4M3 (1440 cols) with the bf16 V row embedded
    at byte offset 1440 of each 1536-B row: halves the AllGather and the
    stage-2 dma_gather traffic vs bf16.
  - Stage 2 msg_A uses fp8 DoubleRow matmuls (two radial parities as the two
    k-tiles), halving PE time; lhsT is a parity-masked fp8 S_w (sw2).
  - Persistent bf16 sw1 serves stage-1 seg-sums and stage-2 msg_B; memory
    term folded into the a1 PSUM group via an identity matmul.
"""
import os
import numpy as np
from math import factorial, pi

import ml_dtypes

import concourse.bacc as bacc
import concourse.bass as bass
import concourse.mybir as mybir
import concourse.tile as tile
from concourse.bass_utils import run_bass_kernel_spmd

# ---- problem constants (hardcoded; must match reference.py) ----
ZS = np.array([1, 6, 7, 8], dtype=np.int64)
NZ = 4
NAB = 3
CHAN = 9
MAX_L = 3
N_RBF = 8
N_RB = 8
CUTOFF = 5.5
MP_NORM = 1.0 / 10.0 ** 0.5
N_NODES = 4000
N_EDGES = 48000

def _make_l_list(max_l):
    lst = []
    for l in range(max_l + 1):
        for lx in range(l, -1, -1):
            for ly in range(l - lx, -1, -1):
                lst.append((lx, ly, l - lx - ly))
    return lst

L_LIST = _make_l_list(MAX_L)
N_L = len(L_LIST)                                   # 20
L_OF = np.array([sum(t) for t in L_LIST])
PREF = np.array([factorial(sum(t)) / (factorial(t[0]) * factorial(t[1]) * factorial(t[2]))
                 for t in L_LIST], dtype=np.float64)
L_RANGES = [(0, 1), (1, 4), (4, 10), (10, 20)]
# batched monomial chain: lists of (out_lo, out_hi, par_lo, comp)
_CHAIN_BATCH = [(4, 7, 1, 0), (7, 9, 2, 1), (9, 10, 3, 2),
                (10, 16, 4, 0), (16, 19, 7, 1), (19, 20, 9, 2)]

NC = 8
NSUB = 32
SUBN = 16
CAP = 192                # edge slots per subtile
ES = NSUB * CAP          # 6144 slots/core
EPB = 128
NBLK = ES // EPB         # 48 blocks/core
NROW = NSUB * SUBN       # 512 node rows/core
TABW = 1536              # fp8 table row: 1440 A + 9 bf16 V + pad (bytes % 256 == 0)
GB = 4                   # subtiles per gather call (6 blocks, 768 idx)
P = 128
F32 = mybir.dt.float32
BF16 = mybir.dt.bfloat16
FP8 = mybir.dt.float8e4
I16 = mybir.dt.int16

_PROGRAM = None


def _block_ranges(s):
    """Blocks + partition ranges covering subtile s's 192 slots."""
    g2 = s // 2
    if s % 2 == 0:
        return [(3 * g2, 0, 128), (3 * g2 + 1, 0, 64)]
    return [(3 * g2 + 1, 64, 128), (3 * g2 + 2, 0, 128)]


# ================= host-side sharding prep (index work only) =================
def _prep(positions, shifts, atomic_numbers, edge_index):
    import heapq
    snd = np.asarray(edge_index[0]).astype(np.int64)
    rcv = np.asarray(edge_index[1]).astype(np.int64)
    an = np.asarray(atomic_numbers)
    species = np.searchsorted(ZS, an)
    indeg = np.bincount(rcv, minlength=N_NODES)
    order = np.argsort(-indeg, kind="stable")
    TS = NC * NSUB
    loads = np.zeros(TS, dtype=np.int64)
    counts = np.zeros(TS, dtype=np.int64)
    assign_sub = np.zeros(N_NODES, dtype=np.int64)
    assign_slot = np.zeros(N_NODES, dtype=np.int64)
    heap = [(0, t) for t in range(TS)]
    heapq.heapify(heap)
    for nd in order:
        pending = []
        while True:
            load, t = heapq.heappop(heap)
            if counts[t] < SUBN:
                break
            pending.append((load, t))
        assign_sub[nd] = t
        assign_slot[nd] = counts[t]
        counts[t] += 1
        loads[t] = load + indeg[nd]
        heapq.heappush(heap, (loads[t], t))
        for it in pending:
            heapq.heappush(heap, it)
    assert loads.max() <= CAP, f"subtile edge overflow: {loads.max()} > {CAP}"

    core_of = assign_sub // NSUB
    sub_of = assign_sub % NSUB
    node_row = core_of * NROW + sub_of * SUBN + assign_slot      # node -> global row
    node_of_row = np.full(NC * NROW, -1, dtype=np.int64)
    node_of_row[node_row] = np.arange(N_NODES)
    # table row in tabfull's (slice, core, 256-row) layout (contiguous AG slices)
    tab_row = (sub_of // 16) * (NC * 256) + core_of * 256 + (sub_of % 16) * SUBN + assign_slot

    e_sub = assign_sub[rcv]
    e_order = np.argsort(e_sub, kind="stable")
    bounds = np.searchsorted(e_sub[e_order], np.arange(TS + 1))

    pos = np.asarray(positions, dtype=np.float32)
    shf = np.asarray(shifts, dtype=np.float32)

    geo = np.zeros((NC, 9, ES), dtype=np.float32)                # [comp(SxyzRxyzShxyz), slot]
    geo[:, 3:6, :] = 1.0                                         # benign pad: R=(1,1,1), S=0
    recvoh = np.zeros((NC, SUBN, ES), dtype=np.float32)
    sendrow = np.zeros((NC, ES), dtype=np.int64)
    for t in range(TS):
        c = t // NSUB; s = t % NSUB
        es = e_order[bounds[t]:bounds[t + 1]]
        k = len(es)
        base = s * CAP
        geo[c, 0:3, base:base + k] = pos[snd[es]].T
        geo[c, 3:6, base:base + k] = pos[rcv[es]].T
        geo[c, 6:9, base:base + k] = shf[es].T
        recvoh[c, assign_slot[rcv[es]], base + np.arange(k)] = 1.0
        sendrow[c, base:base + k] = tab_row[snd[es]]

    # device edge-slot layout: slot -> (blk, p) with slot = blk*128 + p
    def to_pb(a):   # [NC, ..., ES] -> [NC, 128, ..., NBLK]
        a2 = a.reshape(a.shape[:-1] + (NBLK, EPB))               # [..., NBLK, 128]
        return np.moveaxis(a2, -1, 1)                            # [NC, 128, ..., NBLK]

    geo_in = np.ascontiguousarray(to_pb(geo).reshape(NC, P, 9 * NBLK))   # [NC, 128, (comp,blk)]
    # recv one-hot in (blk, n) layout: [NC, 128, NBLK, SUBN]
    recv_in = np.ascontiguousarray(
        to_pb(recvoh).transpose(0, 1, 3, 2).reshape(NC, P, NBLK * SUBN))
    # gather idx: per subtile 192 slots; idx k at partition k%16, col sub*12 + k//16
    gidx = np.zeros((NC, P, NSUB * 12), dtype=np.int16)
    for c in range(NC):
        w = sendrow[c].reshape(NSUB, 12, 16).astype(np.int16)    # [sub, k//16, k%16]
        packed = w.transpose(2, 0, 1).reshape(16, NSUB * 12)     # [k%16, (sub, k//16)]
        for g in range(8):
            gidx[c, g * 16:(g + 1) * 16, :] = packed
    # per-edge-slot sender species (pad -> 0) in device layout [NC, 128, NBLK]
    sendsp = np.zeros((NC, ES), dtype=np.int64)
    for t in range(TS):
        c = t // NSUB; s = t % NSUB
        es = e_order[bounds[t]:bounds[t + 1]]
        sendsp[c, s * CAP:s * CAP + len(es)] = species[snd[es]]
    sendsp_in = to_pb(sendsp)                                    # [NC, 128, NBLK]
    # per-node-row species (empty rows -> 0; all their uses are masked/zero)
    rowsp = np.zeros((NC, NROW), dtype=np.int64)
    msk = node_of_row >= 0
    rowsp.reshape(-1)[msk] = species[node_of_row[msk]]
    return dict(geo=geo_in, recv=recv_in, gidx=gidx, sendsp=sendsp_in, rowsp=rowsp,
                node_of_row=node_of_row, node_row=node_row)


def _consts():
    blkdiag = ((np.arange(P)[:, None] % 16) == (np.arange(P)[None, :] % 16)).astype(np.float32)
    nrow = np.tile((np.arange(1, N_RBF + 1) * pi / CUTOFF).astype(np.float32)[None, :], (P, 1))
    sprow = np.tile(np.sqrt(PREF).astype(np.float32)[None, :], (P, 1))   # [128, 20]
    return np.concatenate([blkdiag, nrow, sprow], axis=1)        # [128, 156]


def _parc16():
    parc = np.zeros((P, 16), dtype=np.float32)                   # [par, r] keep r where r%2==par
    for par in range(2):
        for r in range(8):
            if r % 2 == par:
                parc[:, par * 8 + r] = 1.0
    return parc


# ================= device program =================
def _build(sim_mode=False):
    PH = int(os.environ.get("KPHASES", "63"))  # bit0 base,1 s1,2 node1,3 repack,4 s2,5 node2
    nc = bacc.Bacc("TRN2", target_bir_lowering=False, debug=False,
                   num_devices=(1 if sim_mode else NC),
                   dynamic_dma_scratch_size=32768)
    AF = mybir.ActivationFunctionType
    OP = mybir.AluOpType
    DR = mybir.MatmulPerfMode.DoubleRow

    # x_main packs [geo 432 | cons 156 | wpack 308]
    NMAIN = 9 * NBLK + 156 + 308
    NB16 = NBLK * SUBN + NBLK * NAB + 32     # recv16 768 | embsE16 144 | parc16 16 | zeros 16
    x_main = nc.dram_tensor("x_main", [P, NMAIN], F32, kind="ExternalInput")
    x_b16 = nc.dram_tensor("x_b16", [P, NB16], BF16, kind="ExternalInput")
    x_zero = nc.dram_tensor("x_zero", [P, NBLK * 128], BF16, kind="ExternalInput")
    x_gidx = nc.dram_tensor("x_gidx", [P, NSUB * 12], I16, kind="ExternalInput")
    o_b0 = nc.dram_tensor("o_b0", [P, NSUB * 45], F32, kind="ExternalOutput")
    o_b1 = nc.dram_tensor("o_b1", [P, NSUB * 45], F32, kind="ExternalOutput")

    with tile.TileContext(nc) as tc:
        with (
            tc.tile_pool(name="persist", bufs=1) as pp,
            tc.tile_pool(name="work", bufs=2) as wp,
            tc.tile_pool(name="dram", bufs=1, space="DRAM") as dr,
        ):
            # ---------- loads (geo first so the edge phase starts early) ----------
            main = pp.tile([P, NMAIN], F32)
            C1 = 9 * NBLK
            nc.sync.dma_start(main[:, 0:C1], x_main[:, 0:C1])
            nc.sync.dma_start(main[:, C1:NMAIN], x_main[:, C1:NMAIN])
            b16 = pp.tile([P, NB16], BF16)
            nc.sync.dma_start(b16[:], x_b16[:])
            gidx = pp.tile([P, NSUB * 12], I16)
            nc.sync.dma_start(gidx[:], x_gidx[:])
            o = 0
            geo = main[:, o:o + 9 * NBLK]; o += 9 * NBLK
            cons = main[:, o:o + 156]; o += 156
            wpack = main[:, o:o + 308]; o += 308
            blkdiag = cons[:, 0:128]
            nrow = cons[:, 128:136]
            sprow = cons[:, 136:156]
            EM = wpack[:, 212:308]          # pure emb products
            recv16 = b16[:, 0:NBLK * SUBN]
            embsE16 = b16[:, NBLK * SUBN:NBLK * SUBN + NBLK * NAB]
            parc16 = b16[:, NBLK * SUBN + NBLK * NAB:NBLK * SUBN + NBLK * NAB + 16]
            zeros16 = b16[:, NBLK * SUBN + NBLK * NAB + 16:]

            # ---------- one-time derived weights ----------
            rtl16 = []
            for l in range(MAX_L + 1):
                rtl_t = pp.tile([P, P], BF16, tag=f"rtl{l}")
                rtl16.append(rtl_t)
                nc.vector.tensor_tensor(
                    out=rtl_t[:].rearrange("p (s n) -> p s n", s=8),
                    in0=wpack[:, l * 8:(l + 1) * 8][:, :, None].to_broadcast([P, 8, 16]),
                    in1=blkdiag.rearrange("p (s n) -> p s n", s=8),
                    op=OP.mult)
            WT16 = pp.tile([P, 180], BF16)
            nc.scalar.copy(WT16[:], wpack[:, 32:212])
            ident16 = pp.tile([P, P], BF16)
            nc.scalar.copy(ident16[:], blkdiag[:])

            # ---------- per-edge base phase ----------
            D = pp.tile([P, 3 * NBLK], F32)
            nc.vector.tensor_tensor(out=D[:], in0=geo[:, 3 * NBLK:6 * NBLK],
                                    in1=geo[:, 0:3 * NBLK], op=OP.subtract)
            nc.vector.tensor_tensor(out=D[:], in0=D[:], in1=geo[:, 6 * NBLK:9 * NBLK], op=OP.add)
            sq = wp.tile([P, 3 * NBLK], F32, tag="sq")
            nc.vector.tensor_tensor(out=sq[:], in0=D[:], in1=D[:], op=OP.mult)
            r2 = wp.tile([P, NBLK], F32, tag="r2")
            nc.vector.tensor_tensor(out=r2[:], in0=sq[:, 0:NBLK], in1=sq[:, NBLK:2 * NBLK], op=OP.add)
            nc.vector.tensor_tensor(out=r2[:], in0=r2[:], in1=sq[:, 2 * NBLK:3 * NBLK], op=OP.add)
            rr = wp.tile([P, NBLK], F32, tag="rr")
            nc.scalar.activation(rr[:], r2[:], AF.Sqrt)
            rinv = pp.tile([P, NBLK], F32)
            nc.vector.reciprocal(rinv[:], rr[:])
            uu = wp.tile([P, NBLK], F32, tag="uu")
            nc.vector.tensor_scalar_mul(uu[:], rr[:], 1.0 / CUTOFF)
            U = pp.tile([P, 3 * NBLK], F32)
            nc.vector.tensor_tensor(
                out=U[:].rearrange("p (c b) -> p c b", c=3),
                in0=D[:].rearrange("p (c b) -> p c b", c=3),
                in1=rinv[:, None, :].to_broadcast([P, 3, NBLK]), op=OP.mult)
            # bessel args [128, (blk, r)] + range reduction to [-pi, pi): the
            # reduction runs on gpsimd, in parallel with the DVE angular chain
            arg = wp.tile([P, NBLK * 8], F32, tag="arg")
            nc.vector.tensor_tensor(
                out=arg[:].rearrange("p (b r) -> p b r", r=8),
                in0=rr[:, :, None].to_broadcast([P, NBLK, 8]),
                in1=nrow[:, None, :].to_broadcast([P, NBLK, 8]), op=OP.mult)
            # parallel range reduction: k*2pi with k from 3 independent
            # comparisons (DVE+gpsimd), then one extra fold to [-pi, pi)
            ge1 = wp.tile([P, NBLK * 8], F32, tag="ge1")
            ge2 = wp.tile([P, NBLK * 8], F32, tag="ge2")
            ge3 = wp.tile([P, NBLK * 8], F32, tag="ge3")
            nc.vector.tensor_scalar(out=ge1[:], in0=arg[:], scalar1=float(2 * pi),
                                    scalar2=float(2 * pi), op0=OP.is_ge, op1=OP.mult)
            nc.gpsimd.tensor_scalar(out=ge2[:], in0=arg[:], scalar1=float(4 * pi),
                                    scalar2=float(2 * pi), op0=OP.is_ge, op1=OP.mult)
            nc.vector.tensor_scalar(out=ge3[:], in0=arg[:], scalar1=float(6 * pi),
                                    scalar2=float(2 * pi), op0=OP.is_ge, op1=OP.mult)
            nc.gpsimd.tensor_tensor(out=ge2[:], in0=ge2[:], in1=ge3[:], op=OP.add)
            nc.vector.tensor_tensor(out=arg[:], in0=arg[:], in1=ge1[:], op=OP.subtract)
            nc.vector.tensor_tensor(out=arg[:], in0=arg[:], in1=ge2[:], op=OP.subtract)
            nc.vector.tensor_scalar(out=ge1[:], in0=arg[:], scalar1=float(pi),
                                    scalar2=float(2 * pi), op0=OP.is_ge, op1=OP.mult)
            nc.vector.tensor_tensor(out=arg[:], in0=arg[:], in1=ge1[:], op=OP.subtract)
            # angular monomials ang [128, (blk, i)] scaled by sqrt(PREF), on DVE
            # while gpsimd reduces the bessel arguments
            ang = pp.tile([P, NBLK * N_L], F32)
            angv = ang[:].rearrange("p (b i) -> p b i", i=N_L)
            nc.vector.tensor_scalar(out=angv[:, :, 0], in0=uu[:], scalar1=0.0, scalar2=1.0,
                                    op0=OP.mult, op1=OP.add)
            nc.vector.tensor_copy(
                angv[:, :, 1:4],
                U[:].rearrange("p (c b) -> p b c", c=3))
            for lo, hi, plo, c in _CHAIN_BATCH:
                cnt = hi - lo
                nc.vector.tensor_tensor(
                    out=angv[:, :, lo:hi],
                    in0=angv[:, :, plo:plo + cnt],
                    in1=U[:, c * NBLK:(c + 1) * NBLK][:, :, None].to_broadcast([P, NBLK, cnt]),
                    op=OP.mult)
            # fold the sqrt(PREF) prefactor into the bf16 conversion
            ang16 = pp.tile([P, NBLK * N_L], BF16)
            ang16v = ang16[:].rearrange("p (b i) -> p b i", i=N_L)
            nc.vector.tensor_tensor(
                out=ang16v,
                in0=angv[:],
                in1=sprow[:, None, :].to_broadcast([P, NBLK, N_L]),
                op=OP.mult)

            sinv = wp.tile([P, NBLK * 8], F32, tag="sinv")
            nc.scalar.activation(sinv[:], arg[:], AF.Sin)
            # cutoff polynomial
            u2 = wp.tile([P, NBLK], F32, tag="u2")
            nc.vector.tensor_tensor(out=u2[:], in0=uu[:], in1=uu[:], op=OP.mult)
            a1 = wp.tile([P, NBLK], F32, tag="a1")
            nc.vector.tensor_scalar(out=a1[:], in0=uu[:], scalar1=-48.0, scalar2=28.0,
                                    op0=OP.mult, op1=OP.add)
            g21 = wp.tile([P, NBLK], F32, tag="g21")
            nc.vector.tensor_scalar_mul(g21[:], u2[:], 21.0)
            nc.vector.tensor_tensor(out=g21[:], in0=g21[:], in1=a1[:], op=OP.add)
            u6 = wp.tile([P, NBLK], F32, tag="u6")
            nc.vector.tensor_tensor(out=u6[:], in0=u2[:], in1=u2[:], op=OP.mult)
            nc.vector.tensor_tensor(out=u6[:], in0=u6[:], in1=u2[:], op=OP.mult)
            fc = wp.tile([P, NBLK], F32, tag="fc")
            nc.vector.tensor_tensor(out=fc[:], in0=u6[:], in1=g21[:], op=OP.mult)
            nc.vector.tensor_scalar(out=fc[:], in0=fc[:], scalar1=-1.0, scalar2=1.0,
                                    op0=OP.mult, op1=OP.add)
            lt = wp.tile([P, NBLK], F32, tag="lt")
            nc.vector.tensor_scalar(out=lt[:], in0=uu[:], scalar1=1.0, scalar2=None, op0=OP.is_lt)
            nc.vector.tensor_tensor(out=fc[:], in0=fc[:], in1=lt[:], op=OP.mult)
            scal = wp.tile([P, NBLK], F32, tag="scal")
            nc.vector.tensor_tensor(out=scal[:], in0=rinv[:], in1=fc[:], op=OP.mult)
            nc.vector.tensor_scalar_mul(scal[:], scal[:], float(np.sqrt(2.0 / CUTOFF)))
            # rc in bf16 (single rounding from the f32 product)
            rc16 = pp.tile([P, NBLK * 8], BF16)
            nc.vector.tensor_tensor(
                out=rc16[:].rearrange("p (b r) -> p b r", r=8),
                in0=sinv[:].rearrange("p (b r) -> p b r", r=8),
                in1=scal[:, :, None].to_broadcast([P, NBLK, 8]), op=OP.mult)

            # G1 [128, (blk, i, a)] bf16, per 12-block quarter (all-bf16 2x)
            G1 = pp.tile([P, NBLK * N_L * NAB], BF16)

            def build_g1(g8):
                bs = slice(g8 * 12, g8 * 12 + 12)
                nc.vector.tensor_tensor(
                    out=G1[:].rearrange("p (b i a) -> p b i a", i=N_L, a=NAB)[:, bs],
                    in0=ang16v[:, bs, :, None].to_broadcast([P, 12, N_L, NAB]),
                    in1=embsE16.rearrange("p (b a) -> p b a", a=NAB)[:, bs, None, :].to_broadcast([P, 12, N_L, NAB]),
                    op=OP.mult)

            # sw1 [128, (blk, r, n)] bf16: rc x recv one-hot, per 12-block quarter
            sw1 = pp.tile([P, NBLK * P], BF16)

            def build_sw1(g8):
                bs = slice(g8 * 12, g8 * 12 + 12)
                nc.vector.tensor_tensor(
                    out=sw1[:].rearrange("p (b r n) -> p b r n", r=8, n=16)[:, bs],
                    in0=recv16.rearrange("p (b n) -> p b n", n=SUBN)[:, bs, None, :].to_broadcast([P, 12, 8, 16]),
                    in1=rc16[:].rearrange("p (b r) -> p b r", r=8)[:, bs, :, None].to_broadcast([P, 12, 8, 16]),
                    op=OP.mult)

            # parity-split S_w in fp8 for stage-2 DoubleRow msg_A. The zero
            # half is DMA-broadcast-filled (off-engine); only the 6144
            # nonzeros (at r = 2*rp + q, a linear-stride AP) are computed.
            sw2 = pp.tile([P, NBLK * 256], FP8)
            nc.sync.dma_start(out=sw2[:].bitcast(BF16), in_=x_zero[:])
            sw2nz = sw2[:].rearrange("p (b q rp par n) -> p b q rp par n",
                                     q=2, rp=4, par=2, n=16)
            rc16r = rc16[:].rearrange("p (b rp par) -> p b rp par", rp=4, par=2)

            def build_sw2(q, half, eng):
                bs = slice(half * 24, half * 24 + 24)
                eng.tensor_tensor(
                    out=sw2nz[:, bs, q, :, q, :],
                    in0=recv16.rearrange("p (b n) -> p b n", n=SUBN)[:, bs, None, :].to_broadcast([P, 24, 4, 16]),
                    in1=rc16r[:, bs, :, q][:, :, :, None].to_broadcast([P, 24, 4, 16]),
                    op=OP.mult)

            def symmetrize_pool(bv, sv, ns):
                # bv [P,ns,5,c]; sv [P,ns,20,c]: sum-of-squares tree on gpsimd
                # (bv[:,:,0,:] filled by the caller via Act copy)
                s5 = wp.tile([P, 8 * 5 * CHAN], F32, tag="ps5")
                v5 = s5[:].rearrange("p (s i c) -> p s i c", i=5, c=CHAN)[:, 0:ns]
                s3 = wp.tile([P, 8 * 3 * CHAN], F32, tag="ps3")
                v3 = s3[:].rearrange("p (s i c) -> p s i c", i=3, c=CHAN)[:, 0:ns]
                TT = nc.gpsimd.tensor_tensor
                nc.gpsimd.tensor_copy(bv[:, :, 1, :], sv[:, :, 0, :])
                # l=1: i 1..4
                TT(out=v3[:, :, 0, :], in0=sv[:, :, 1, :], in1=sv[:, :, 2, :], op=OP.add)
                TT(out=bv[:, :, 2, :], in0=v3[:, :, 0, :], in1=sv[:, :, 3, :], op=OP.add)
                # l=2: i 4..10
                TT(out=v3[:], in0=sv[:, :, 4:7, :], in1=sv[:, :, 7:10, :], op=OP.add)
                TT(out=v5[:, :, 0, :], in0=v3[:, :, 0, :], in1=v3[:, :, 1, :], op=OP.add)
                TT(out=bv[:, :, 3, :], in0=v5[:, :, 0, :], in1=v3[:, :, 2, :], op=OP.add)
                # l=3: i 10..20
                TT(out=v5[:], in0=sv[:, :, 10:15, :], in1=sv[:, :, 15:20, :], op=OP.add)
                TT(out=v3[:, :, 0:2, :], in0=v5[:, :, 0:2, :], in1=v5[:, :, 2:4, :], op=OP.add)
                TT(out=v3[:, :, 2, :], in0=v3[:, :, 0, :], in1=v3[:, :, 1, :], op=OP.add)
                TT(out=bv[:, :, 4, :], in0=v3[:, :, 2, :], in1=v5[:, :, 4, :], op=OP.add)

            A16 = pp.tile([P, NSUB * 180], BF16)
            A8 = pp.tile([P, NSUB * 180], FP8)
            mem16 = pp.tile([P, NSUB * 180], BF16)
            B0_all = pp.tile([P, NSUB * 45], F32)
            B1_all = pp.tile([P, NSUB * 45], F32)
            red1 = pp.tile([P, NSUB * CHAN], F32)
            chic = pp.tile([16, NSUB * CHAN], F32)
            Vsb = pp.tile([16, NSUB * CHAN], BF16)

            tabsh = dr.tile([NROW, TABW], FP8)
            tabfull = dr.tile([NC * NROW, TABW], FP8)

            # ---------- stage 1: seg-sum + RT for all 4 groups first (PE/Act
            # critical path unblocked), node-level phases stream behind ----------
            s1ctx = tc.tile_pool(name="ps_s1", bufs=2, space="PSUM")
            ps_s1 = s1ctx.__enter__()
            t1gs = []
            if PH & 2:
                build_g1(0)
                build_sw1(0)
            for g8 in range(4 if (PH & 2) else 0):
                if g8 < 3:
                    build_g1(g8 + 1)
                    build_sw1(g8 + 1)
                t1g = ps_s1.tile([P, 480], F32, space="PSUM", tag="t1g", bufs=4)
                t1gs.append(t1g)
                pend = []
                for j in range(9):
                    if j < 8:
                        s = g8 * 8 + j
                        t0 = ps_s1.tile([P, 60], F32, space="PSUM", tag="t0", bufs=3)
                        ranges = _block_ranges(s)
                        for mi, (blk, p0, p1) in enumerate(ranges):
                            nc.tensor.matmul(t0[:], lhsT=sw1[p0:p1, blk * 128:(blk + 1) * 128],
                                             rhs=G1[p0:p1, blk * 60:(blk + 1) * 60],
                                             start=(mi == 0), stop=(mi == len(ranges) - 1))
                        t0c = wp.tile([P, 60], BF16, tag="t0c", bufs=4)
                        nc.scalar.copy(t0c[:], t0[:])
                        pend.append((j, t0c))
                    if (j > 0 or g8 > 0) and pend and (j == 8 or len(pend) > 1):
                        jj, t0cp = pend.pop(0)
                        for l, (a, b) in enumerate(L_RANGES):
                            nc.tensor.matmul(
                                t1g[:, jj * 60 + a * NAB: jj * 60 + b * NAB],
                                lhsT=rtl16[l][:], rhs=t0cp[:, a * NAB:b * NAB],
                                start=True, stop=True)
            for g8 in range(4 if (PH & 4) else 0):
                # ---- group node-level: A16, A8, B0, chi, V, repack, AG slice ----
                t1g = t1gs[g8]
                sl = slice(g8 * 1440, (g8 + 1) * 1440)
                sl45 = slice(g8 * 360, (g8 + 1) * 360)
                sl9 = slice(g8 * 72, (g8 + 1) * 72)
                # A16 holds the unscaled A (bf16); MP_NORM enters only in the
                # fp8 A8 copy (table) and chic
                nc.vector.tensor_tensor(
                    out=A16[:, sl].rearrange("p (j ia b) -> p j ia b", j=8, b=NAB),
                    in0=t1g[:].rearrange("p (j ia) -> p j ia", j=8)[:, :, :, None].to_broadcast([P, 8, 60, NAB]),
                    in1=EM[:, g8 * 24:(g8 + 1) * 24].rearrange("p (j b) -> p j b", b=NAB)[:, :, None, :].to_broadcast([P, 8, 60, NAB]),
                    op=OP.mult)
                nc.scalar.activation(A8[:, sl], A16[:, sl], AF.Copy, scale=float(MP_NORM))
                scr = wp.tile([P, 1440], F32, tag="scr")
                nc.scalar.activation(scr[:], A16[:, sl], AF.Square)
                bv = B0_all[:, sl45].rearrange("p (s l c) -> p s l c", l=5, c=CHAN)
                sv = scr[:].rearrange("p (s i c) -> p s i c", i=N_L, c=CHAN)
                nc.gpsimd.tensor_copy(
                    bv[:, :, 0, :],
                    A16[:, sl].rearrange("p (s i c) -> p s i c", i=N_L, c=CHAN)[:, :, 0, :])
                rv = red1[:, sl9].rearrange("p (s c) -> p s c", c=CHAN)
                if g8 == 3:
                    # last group: chi computed straight from the squares so the
                    # final AG slice doesn't wait on the symmetrize tree
                    nc.vector.tensor_reduce(
                        out=rv, in_=sv.transpose([0, 1, 3, 2]),
                        axis=mybir.AxisListType.X, op=OP.add)
                    nc.vector.tensor_tensor(out=rv, in0=rv, in1=bv[:, :, 0, :], op=OP.add)
                    symmetrize_pool(bv, sv, 8)
                else:
                    symmetrize_pool(bv, sv, 8)
                    nc.vector.tensor_reduce(
                        out=rv, in_=bv.transpose([0, 1, 3, 2]),
                        axis=mybir.AxisListType.X, op=OP.add)
                chips = ps_s1.tile([16, 72], F32, space="PSUM", tag="chips", bufs=1)
                nc.tensor.matmul(chips[:], lhsT=blkdiag[:, 0:16], rhs=red1[:, sl9],
                                 start=True, stop=True)
                nc.vector.tensor_scalar_mul(chic[:, sl9], chips[:], float(MP_NORM))
                nc.vector.tensor_tensor(
                    out=Vsb[:, sl9].rearrange("p (s a b) -> p s a b", a=NAB, b=NAB),
                    in0=chic[:, sl9].rearrange("p (s a b) -> p s a b", a=NAB, b=NAB),
                    in1=EM[0:16, g8 * 24:(g8 + 1) * 24].rearrange("p (s a) -> p s a", a=NAB)[:, :, :, None].to_broadcast([16, 8, NAB, NAB]),
                    op=OP.mult)
                if (PH & 8) and g8 % 2 == 1:
                    # repack super-group: A rows (fp8) + V column (bf16 in the
                    # row pad) for 16 subtiles (256 table rows); then AG slice
                    sg = g8 // 2
                    ssl = slice(sg * 2880, (sg + 1) * 2880)
                    ssl9 = slice(sg * 144, (sg + 1) * 144)
                    for sp in range(8):
                        nc.sync.dma_start(
                            out=tabsh[:].rearrange("(s n) w -> n s w", n=SUBN)[:, sg * 16:(sg + 1) * 16, sp * 180:(sp + 1) * 180],
                            in_=A8[sp * 16:(sp + 1) * 16, ssl].rearrange("n (s f) -> n s f", f=180))
                    nc.sync.dma_start(
                        out=tabsh[:].rearrange("(s n) w -> n s w", n=SUBN)[:, sg * 16:(sg + 1) * 16, 1440:1458].bitcast(BF16),
                        in_=Vsb[:, ssl9].rearrange("n (s c) -> n s c", c=CHAN))
                    rsl = slice(sg * 256, (sg + 1) * 256)
                    if sim_mode:
                        # stand-in for the sliced AllGather: 4 local copies per
                        # slice model the 8-core AG of the 0.77MB/rank fp8
                        # shard (same total bytes as the real collective)
                        for _cc in range(4):
                            nc.sync.dma_start(
                                tabfull[sg * NC * 256 + _cc * 256:
                                        sg * NC * 256 + (_cc + 1) * 256, :],
                                tabsh[rsl, :])
                    else:
                        # tabfull rows are (slice, core, 256): slice output is
                        # the contiguous rank-major block for this slice
                        nc.gpsimd.collective_compute(
                            "AllGather", mybir.AluOpType.bypass,
                            replica_groups=[list(range(NC))],
                            ins=[tabsh[rsl, :]],
                            outs=[tabfull[sg * NC * 256:(sg + 1) * NC * 256, :]])
            s1ctx.__exit__(None, None, None)

            # sw2 fp8 build: fills the AllGather window
            if PH & 16:
                build_sw2(0, 0, nc.vector)
                build_sw2(1, 0, nc.gpsimd)
                build_sw2(0, 1, nc.vector)
                build_sw2(1, 1, nc.vector)
            sw2v = sw2[:].rearrange("p (b q rn) -> p b q rn", q=2, rn=128)
            # memory term (bf16 fast path; WT pre-divided by MP_NORM on host);
            # emitted late so it lands in the AllGather/stage-2 DVE idle time
            nc.vector.tensor_tensor(
                out=mem16[:].rearrange("p (s f) -> p s f", f=180),
                in0=A16[:].rearrange("p (s f) -> p s f", f=180),
                in1=WT16[:, None, :].to_broadcast([P, NSUB, 180]),
                op=OP.mult)

            # ---------- stage 2 (1-deep software pipeline: pair gg's
            # gather-independent matmuls are emitted before pair gg-1's
            # t2s-dependent tail, so PE never stalls on the DVE hop) ----------
            s2ctx = tc.tile_pool(name="ps_s2", bufs=2, space="PSUM")
            ps_s2 = s2ctx.__enter__()
            gat4 = None
            NP = NSUB // 2 if (PH & 16) else 0
            hist = {}
            for gg in range(NP + 1 if NP else 0):
                if gg < NP:
                    if gg % 2 == 0:
                        g4 = gg // 2
                        gat4 = wp.tile([P, 6, TABW], FP8, tag="gat", bufs=3)
                        nc.gpsimd.dma_gather(gat4[:], tabfull[:],
                                             gidx[:, g4 * 48:(g4 + 1) * 48],
                                             GB * CAP, GB * CAP, TABW)
                    b3 = (gg % 2) * 3             # this pair's blocks within gat4
                    gatv = gat4[:, b3:b3 + 3, 1440:1458].bitcast(BF16)
                    G2 = wp.tile([P, 3, 180], BF16, tag="g2", bufs=3)
                    nc.vector.tensor_tensor(
                        out=G2[:].rearrange("p b (i c) -> p b i c", c=CHAN),
                        in0=ang16v[:, 3 * gg:3 * gg + 3, :, None].to_broadcast([P, 3, N_L, CHAN]),
                        in1=gatv[:, :, None, :].to_broadcast([P, 3, N_L, CHAN]),
                        op=OP.mult)
                    t2pair = ps_s2.tile([P, 360], F32, space="PSUM", tag="t2", bufs=3)
                    a1pair = ps_s2.tile([P, 360], F32, space="PSUM", tag="a1p", bufs=3)
                    hist[gg] = (t2pair, a1pair)
                    for s2 in range(2):
                        s = gg * 2 + s2
                        osl = slice(s2 * 180, (s2 + 1) * 180)
                        ranges = _block_ranges(s)
                        for mi, (blk, p0, p1) in enumerate(ranges):
                            bloc = blk - 3 * gg + b3
                            nc.tensor.matmul(
                                t2pair[:, osl],
                                lhsT=sw1[p0:p1, blk * 128:(blk + 1) * 128],
                                rhs=G2[p0:p1, bloc - b3, :],
                                start=(mi == 0), stop=(mi == len(ranges) - 1))
                        # msg_A: fp8 DoubleRow, radial parity pair as the k-tiles
                        for ri, (blk, p0, p1) in enumerate(ranges):
                            bloc = blk - 3 * gg + b3
                            for k in range(4):
                                nc.tensor.matmul(
                                    a1pair[k * 32:(k + 1) * 32, osl],
                                    lhsT=sw2v[p0:p1, blk, :, k * 32:(k + 1) * 32],
                                    rhs=gat4[p0:p1, bloc, 2 * k * 180:(2 * k + 2) * 180].rearrange(
                                        "p (q f) -> p q f", q=2),
                                    start=(ri == 0), stop=False,
                                    perf_mode=DR, tile_position=(p0, k * 32))
                if gg == 0:
                    continue
                gp = gg - 1
                t2pair, a1pair = hist.pop(gp)
                t2s = wp.tile([P, 360], BF16, tag="t2s", bufs=3)
                for s2 in range(2):
                    s = gp * 2 + s2
                    osl = slice(s2 * 180, (s2 + 1) * 180)
                    nc.vector.tensor_tensor(
                        out=t2s[:, osl].rearrange("p (i a b) -> p i a b", a=NAB, b=NAB),
                        in0=t2pair[:, osl].rearrange("p (i a b) -> p i a b", a=NAB, b=NAB),
                        in1=EM[:, s * NAB:(s + 1) * NAB][:, None, None, :].to_broadcast([P, N_L, NAB, NAB]),
                        op=OP.mult)
                    for l, (a, b) in enumerate(L_RANGES):
                        nc.tensor.matmul(
                            a1pair[:, s2 * 180 + a * CHAN: s2 * 180 + b * CHAN],
                            lhsT=rtl16[l][:], rhs=t2s[:, s2 * 180 + a * CHAN: s2 * 180 + b * CHAN],
                            start=False, stop=False)
                    # fold the memory term into the PSUM group (identity matmul)
                    nc.tensor.matmul(
                        a1pair[:, osl], lhsT=ident16[:],
                        rhs=mem16[:, s * 180:(s + 1) * 180],
                        start=False, stop=True)
                if not (PH & 32):
                    continue
                # ---- stage 2 node-level per pair: B1 + output ----
                sl45 = slice(gp * 90, (gp + 1) * 90)
                bv = B1_all[:, sl45].rearrange("p (s l c) -> p s l c", l=5, c=CHAN)
                a1v = a1pair[:].rearrange("p (s i c) -> p s i c", i=N_L, c=CHAN)
                scr1 = wp.tile([P, 360], F32, tag="scr1", bufs=3)
                nc.scalar.activation(scr1[:], a1pair[:], AF.Square)
                nc.scalar.copy(bv[:, :, 0, :], a1v[:, :, 0, :])
                sv = scr1[:].rearrange("p (s i c) -> p s i c", i=N_L, c=CHAN)
                for l, (a, b) in enumerate(L_RANGES):
                    nc.vector.tensor_reduce(
                        out=bv[:, :, l + 1, :],
                        in_=sv[:, :, a:b, :].transpose([0, 1, 3, 2]),
                        axis=mybir.AxisListType.X, op=OP.add)
                nc.sync.dma_start(o_b1[:, sl45], B1_all[:, sl45])
            s2ctx.__exit__(None, None, None)
            nc.sync.dma_start(o_b0[:], B0_all[:])
            if not (PH & 4):
                nc.sync.dma_start(o_b0[:, 0:156], cons[:])
            if not (PH & 32):
                nc.sync.dma_start(o_b1[:, 0:156], cons[:])
    nc.compile()
    return nc


# ================= public entry =================
def kernel(positions, shifts, W_emb, W_rt, W_nm, atomic_numbers, edge_index):
    global _PROGRAM
    prep = _prep(positions, shifts, atomic_numbers, edge_index)
    consts = _consts()
    if _PROGRAM is None:
        _PROGRAM = _build()
    nc = _PROGRAM
    wemb = np.asarray(W_emb, dtype=np.float32)
    wrt = np.asarray(W_rt, dtype=np.float32)
    wnm = np.asarray(W_nm, dtype=np.float32)
    # host-replicated weight patterns (pure tiling/gathers of the small weights)
    pg = np.arange(P) // 16                                   # r|s' group per partition
    rtlw = wrt[:, pg, :].transpose(1, 0, 2).reshape(P, 32)    # [p, (l, s')] = W_rt[l, p//16, s']
    wtp = wnm[0, pg][:, L_OF, :].reshape(P, 180)
    parc = _parc16()
    in_maps = []
    for c in range(NC):
        em = wemb[prep["rowsp"][c].reshape(NSUB, SUBN)]       # [sub, n, a]
        em = em[:, np.arange(P) % 16, :].transpose(1, 0, 2).reshape(P, NSUB * NAB)
        wpack = np.concatenate([rtlw, wtp, em], axis=1).astype(np.float32)
        embse = wemb[prep["sendsp"][c]].reshape(P, NBLK * NAB).astype(np.float32)
        main = np.ascontiguousarray(np.concatenate(
            [prep["geo"][c], consts, wpack],
            axis=1).astype(np.float32))
        b16 = np.ascontiguousarray(np.concatenate(
            [prep["recv"][c], embse, parc, np.zeros((P, 16), np.float32)],
            axis=1).astype(ml_dtypes.bfloat16))
        in_maps.append(dict(x_main=main, x_b16=b16, x_gidx=prep["gidx"][c],
                            x_zero=np.zeros((P, NBLK * 128), ml_dtypes.bfloat16)))
    res = run_bass_kernel_spmd(nc, in_maps, list(range(NC))).results
    # unshard: [128=(s',n), (sub, l, c)] -> node rows
    out = np.zeros((N_NODES, N_RB, 5, CHAN, 2), dtype=np.float32)
    node_of_row = prep["node_of_row"]
    for c in range(NC):
        for mp, name in ((0, "o_b0"), (1, "o_b1")):
            arr = np.asarray(res[c][name], dtype=np.float32).reshape(8, SUBN, NSUB, 5, CHAN)
            rows = arr.transpose(2, 1, 0, 3, 4).reshape(NROW, N_RB, 5, CHAN)
            valid = node_of_row[c * NROW:(c + 1) * NROW] >= 0
            out[node_of_row[c * NROW:(c + 1) * NROW][valid], :, :, :, mp] = rows[valid]
    return out


# revision 27
# speedup vs baseline: 1.0837x; 1.0284x over previous
"""Trainium2 Bass kernel for the CACE message-passing GNN (nn_Cace_58291296141968).

Strategy (8 NeuronCores, SPMD), v3:
  - Receivers load-balanced onto 8 cores x 32 subtiles x 16 node slots; edges
    padded to CAP=192 slots/subtile (48 blocks of 128 slots per core).
  - sqrt(multinomial-prefactor) folded into the angular monomials; MP_NORM
    folded into the node table (A rows and V).
  - Node A table stored in FP8-E4M3 (1440 cols) with the bf16 V row embedded
    at byte offset 1440 of each 1536-B row: halves the AllGather and the
    stage-2 dma_gather traffic vs bf16.
  - Stage 2 msg_A uses fp8 DoubleRow matmuls (two radial parities as the two
    k-tiles), halving PE time; lhsT is a parity-masked fp8 S_w (sw2).
  - Persistent bf16 sw1 serves stage-1 seg-sums and stage-2 msg_B; memory
    term folded into the a1 PSUM group via an identity matmul.
"""
import os
import numpy as np
from math import factorial, pi

import ml_dtypes

import concourse.bacc as bacc
import concourse.bass as bass
import concourse.mybir as mybir
import concourse.tile as tile
from concourse.bass_utils import run_bass_kernel_spmd

# ---- problem constants (hardcoded; must match reference.py) ----
ZS = np.array([1, 6, 7, 8], dtype=np.int64)
NZ = 4
NAB = 3
CHAN = 9
MAX_L = 3
N_RBF = 8
N_RB = 8
CUTOFF = 5.5
MP_NORM = 1.0 / 10.0 ** 0.5
N_NODES = 4000
N_EDGES = 48000

def _make_l_list(max_l):
    lst = []
    for l in range(max_l + 1):
        for lx in range(l, -1, -1):
            for ly in range(l - lx, -1, -1):
                lst.append((lx, ly, l - lx - ly))
    return lst

L_LIST = _make_l_list(MAX_L)
N_L = len(L_LIST)                                   # 20
L_OF = np.array([sum(t) for t in L_LIST])
PREF = np.array([factorial(sum(t)) / (factorial(t[0]) * factorial(t[1]) * factorial(t[2]))
                 for t in L_LIST], dtype=np.float64)
L_RANGES = [(0, 1), (1, 4), (4, 10), (10, 20)]
# batched monomial chain: lists of (out_lo, out_hi, par_lo, comp)
_CHAIN_BATCH = [(4, 7, 1, 0), (7, 9, 2, 1), (9, 10, 3, 2),
                (10, 16, 4, 0), (16, 19, 7, 1), (19, 20, 9, 2)]

NC = 8
NSUB = 32
SUBN = 16
CAP = 192                # edge slots per subtile
ES = NSUB * CAP          # 6144 slots/core
EPB = 128
NBLK = ES // EPB         # 48 blocks/core
NROW = NSUB * SUBN       # 512 node rows/core
TABW = 1536              # fp8 table row: 1440 A + 9 bf16 V + pad (bytes % 256 == 0)
GB = 4                   # subtiles per gather call (6 blocks, 768 idx)
P = 128
F32 = mybir.dt.float32
BF16 = mybir.dt.bfloat16
FP8 = mybir.dt.float8e4
I16 = mybir.dt.int16

_PROGRAM = None


def _block_ranges(s):
    """Blocks + partition ranges covering subtile s's 192 slots."""
    g2 = s // 2
    if s % 2 == 0:
        return [(3 * g2, 0, 128), (3 * g2 + 1, 0, 64)]
    return [(3 * g2 + 1, 64, 128), (3 * g2 + 2, 0, 128)]


# ================= host-side sharding prep (index work only) =================
def _prep(positions, shifts, atomic_numbers, edge_index):
    import heapq
    snd = np.asarray(edge_index[0]).astype(np.int64)
    rcv = np.asarray(edge_index[1]).astype(np.int64)
    an = np.asarray(atomic_numbers)
    species = np.searchsorted(ZS, an)
    indeg = np.bincount(rcv, minlength=N_NODES)
    order = np.argsort(-indeg, kind="stable")
    TS = NC * NSUB
    loads = np.zeros(TS, dtype=np.int64)
    counts = np.zeros(TS, dtype=np.int64)
    assign_sub = np.zeros(N_NODES, dtype=np.int64)
    assign_slot = np.zeros(N_NODES, dtype=np.int64)
    heap = [(0, t) for t in range(TS)]
    heapq.heapify(heap)
    for nd in order:
        pending = []
        while True:
            load, t = heapq.heappop(heap)
            if counts[t] < SUBN:
                break
            pending.append((load, t))
        assign_sub[nd] = t
        assign_slot[nd] = counts[t]
        counts[t] += 1
        loads[t] = load + indeg[nd]
        heapq.heappush(heap, (loads[t], t))
        for it in pending:
            heapq.heappush(heap, it)
    assert loads.max() <= CAP, f"subtile edge overflow: {loads.max()} > {CAP}"

    core_of = assign_sub // NSUB
    sub_of = assign_sub % NSUB
    node_row = core_of * NROW + sub_of * SUBN + assign_slot      # node -> global row
    node_of_row = np.full(NC * NROW, -1, dtype=np.int64)
    node_of_row[node_row] = np.arange(N_NODES)
    # table row in tabfull's (slice, core, 256-row) layout (contiguous AG slices)
    tab_row = (sub_of // 16) * (NC * 256) + core_of * 256 + (sub_of % 16) * SUBN + assign_slot

    e_sub = assign_sub[rcv]
    e_order = np.argsort(e_sub, kind="stable")
    bounds = np.searchsorted(e_sub[e_order], np.arange(TS + 1))

    pos = np.asarray(positions, dtype=np.float32)
    shf = np.asarray(shifts, dtype=np.float32)

    geo = np.zeros((NC, 9, ES), dtype=np.float32)                # [comp(SxyzRxyzShxyz), slot]
    geo[:, 3:6, :] = 1.0                                         # benign pad: R=(1,1,1), S=0
    recvoh = np.zeros((NC, SUBN, ES), dtype=np.float32)
    sendrow = np.zeros((NC, ES), dtype=np.int64)
    for t in range(TS):
        c = t // NSUB; s = t % NSUB
        es = e_order[bounds[t]:bounds[t + 1]]
        k = len(es)
        base = s * CAP
        geo[c, 0:3, base:base + k] = pos[snd[es]].T
        geo[c, 3:6, base:base + k] = pos[rcv[es]].T
        geo[c, 6:9, base:base + k] = shf[es].T
        recvoh[c, assign_slot[rcv[es]], base + np.arange(k)] = 1.0
        sendrow[c, base:base + k] = tab_row[snd[es]]

    # device edge-slot layout: slot -> (blk, p) with slot = blk*128 + p
    def to_pb(a):   # [NC, ..., ES] -> [NC, 128, ..., NBLK]
        a2 = a.reshape(a.shape[:-1] + (NBLK, EPB))               # [..., NBLK, 128]
        return np.moveaxis(a2, -1, 1)                            # [NC, 128, ..., NBLK]

    geo_in = np.ascontiguousarray(to_pb(geo).reshape(NC, P, 9 * NBLK))   # [NC, 128, (comp,blk)]
    # recv one-hot in (blk, n) layout: [NC, 128, NBLK, SUBN]
    recv_in = np.ascontiguousarray(
        to_pb(recvoh).transpose(0, 1, 3, 2).reshape(NC, P, NBLK * SUBN))
    # gather idx: per subtile 192 slots; idx k at partition k%16, col sub*12 + k//16
    gidx = np.zeros((NC, P, NSUB * 12), dtype=np.int16)
    for c in range(NC):
        w = sendrow[c].reshape(NSUB, 12, 16).astype(np.int16)    # [sub, k//16, k%16]
        packed = w.transpose(2, 0, 1).reshape(16, NSUB * 12)     # [k%16, (sub, k//16)]
        for g in range(8):
            gidx[c, g * 16:(g + 1) * 16, :] = packed
    # per-edge-slot sender species (pad -> 0) in device layout [NC, 128, NBLK]
    sendsp = np.zeros((NC, ES), dtype=np.int64)
    for t in range(TS):
        c = t // NSUB; s = t % NSUB
        es = e_order[bounds[t]:bounds[t + 1]]
        sendsp[c, s * CAP:s * CAP + len(es)] = species[snd[es]]
    sendsp_in = to_pb(sendsp)                                    # [NC, 128, NBLK]
    # per-node-row species (empty rows -> 0; all their uses are masked/zero)
    rowsp = np.zeros((NC, NROW), dtype=np.int64)
    msk = node_of_row >= 0
    rowsp.reshape(-1)[msk] = species[node_of_row[msk]]
    return dict(geo=geo_in, recv=recv_in, gidx=gidx, sendsp=sendsp_in, rowsp=rowsp,
                node_of_row=node_of_row, node_row=node_row)


def _consts():
    blkdiag = ((np.arange(P)[:, None] % 16) == (np.arange(P)[None, :] % 16)).astype(np.float32)
    nrow = np.tile((np.arange(1, N_RBF + 1) * pi / CUTOFF).astype(np.float32)[None, :], (P, 1))
    sprow = np.tile(np.sqrt(PREF).astype(np.float32)[None, :], (P, 1))   # [128, 20]
    return np.concatenate([blkdiag, nrow, sprow], axis=1)        # [128, 156]


def _parc16():
    parc = np.zeros((P, 16), dtype=np.float32)                   # [par, r] keep r where r%2==par
    for par in range(2):
        for r in range(8):
            if r % 2 == par:
                parc[:, par * 8 + r] = 1.0
    return parc


# ================= device program =================
def _build(sim_mode=False):
    PH = int(os.environ.get("KPHASES", "63"))  # bit0 base,1 s1,2 node1,3 repack,4 s2,5 node2
    nc = bacc.Bacc("TRN2", target_bir_lowering=False, debug=False,
                   num_devices=(1 if sim_mode else NC),
                   dynamic_dma_scratch_size=32768)
    AF = mybir.ActivationFunctionType
    OP = mybir.AluOpType
    DR = mybir.MatmulPerfMode.DoubleRow

    # x_main packs [geo 432 | cons 156 | wpack 308]
    NMAIN = 9 * NBLK + 156 + 308
    NB16 = NBLK * SUBN + NBLK * NAB + 32     # recv16 768 | embsE16 144 | parc16 16 | zeros 16
    x_main = nc.dram_tensor("x_main", [P, NMAIN], F32, kind="ExternalInput")
    x_b16 = nc.dram_tensor("x_b16", [P, NB16], BF16, kind="ExternalInput")
    x_zero = nc.dram_tensor("x_zero", [P, NBLK * 128], BF16, kind="ExternalInput")
    x_gidx = nc.dram_tensor("x_gidx", [P, NSUB * 12], I16, kind="ExternalInput")
    o_b0 = nc.dram_tensor("o_b0", [P, NSUB * 45], F32, kind="ExternalOutput")
    o_b1 = nc.dram_tensor("o_b1", [P, NSUB * 45], F32, kind="ExternalOutput")

    with tile.TileContext(nc) as tc:
        with (
            tc.tile_pool(name="persist", bufs=1) as pp,
            tc.tile_pool(name="work", bufs=2) as wp,
            tc.tile_pool(name="dram", bufs=1, space="DRAM") as dr,
        ):
            # ---------- loads (geo first so the edge phase starts early) ----------
            main = pp.tile([P, NMAIN], F32)
            C1 = 9 * NBLK
            nc.sync.dma_start(main[:, 0:C1], x_main[:, 0:C1])
            nc.sync.dma_start(main[:, C1:NMAIN], x_main[:, C1:NMAIN])
            b16 = pp.tile([P, NB16], BF16)
            nc.sync.dma_start(b16[:], x_b16[:])
            gidx = pp.tile([P, NSUB * 12], I16)
            nc.sync.dma_start(gidx[:], x_gidx[:])
            o = 0
            geo = main[:, o:o + 9 * NBLK]; o += 9 * NBLK
            cons = main[:, o:o + 156]; o += 156
            wpack = main[:, o:o + 308]; o += 308
            blkdiag = cons[:, 0:128]
            nrow = cons[:, 128:136]
            sprow = cons[:, 136:156]
            EM = wpack[:, 212:308]          # pure emb products
            recv16 = b16[:, 0:NBLK * SUBN]
            embsE16 = b16[:, NBLK * SUBN:NBLK * SUBN + NBLK * NAB]
            parc16 = b16[:, NBLK * SUBN + NBLK * NAB:NBLK * SUBN + NBLK * NAB + 16]
            zeros16 = b16[:, NBLK * SUBN + NBLK * NAB + 16:]

            # ---------- one-time derived weights ----------
            rtl16 = []
            for l in range(MAX_L + 1):
                rtl_t = pp.tile([P, P], BF16, tag=f"rtl{l}")
                rtl16.append(rtl_t)
                nc.vector.tensor_tensor(
                    out=rtl_t[:].rearrange("p (s n) -> p s n", s=8),
                    in0=wpack[:, l * 8:(l + 1) * 8][:, :, None].to_broadcast([P, 8, 16]),
                    in1=blkdiag.rearrange("p (s n) -> p s n", s=8),
                    op=OP.mult)
            # ---------- per-edge base phase ----------
            D = pp.tile([P, 3 * NBLK], F32)
            nc.vector.tensor_tensor(out=D[:], in0=geo[:, 3 * NBLK:6 * NBLK],
                                    in1=geo[:, 0:3 * NBLK], op=OP.subtract)
            nc.vector.tensor_tensor(out=D[:], in0=D[:], in1=geo[:, 6 * NBLK:9 * NBLK], op=OP.add)
            sq = wp.tile([P, 3 * NBLK], F32, tag="sq")
            nc.vector.tensor_tensor(out=sq[:], in0=D[:], in1=D[:], op=OP.mult)
            r2 = wp.tile([P, NBLK], F32, tag="r2")
            nc.vector.tensor_tensor(out=r2[:], in0=sq[:, 0:NBLK], in1=sq[:, NBLK:2 * NBLK], op=OP.add)
            nc.vector.tensor_tensor(out=r2[:], in0=r2[:], in1=sq[:, 2 * NBLK:3 * NBLK], op=OP.add)
            rr = wp.tile([P, NBLK], F32, tag="rr")
            nc.scalar.activation(rr[:], r2[:], AF.Sqrt)
            rinv = pp.tile([P, NBLK], F32)
            nc.vector.reciprocal(rinv[:], rr[:])
            uu = wp.tile([P, NBLK], F32, tag="uu")
            nc.vector.tensor_scalar_mul(uu[:], rr[:], 1.0 / CUTOFF)
            U = pp.tile([P, 3 * NBLK], F32)
            nc.vector.tensor_tensor(
                out=U[:].rearrange("p (c b) -> p c b", c=3),
                in0=D[:].rearrange("p (c b) -> p c b", c=3),
                in1=rinv[:, None, :].to_broadcast([P, 3, NBLK]), op=OP.mult)
            # bessel args [128, (blk, r)] + range reduction to [-pi, pi): the
            # reduction runs on gpsimd, in parallel with the DVE angular chain
            arg = wp.tile([P, NBLK * 8], F32, tag="arg")
            nc.vector.tensor_tensor(
                out=arg[:].rearrange("p (b r) -> p b r", r=8),
                in0=rr[:, :, None].to_broadcast([P, NBLK, 8]),
                in1=nrow[:, None, :].to_broadcast([P, NBLK, 8]), op=OP.mult)
            # parallel range reduction: k*2pi with k from 3 independent
            # comparisons (DVE+gpsimd), then one extra fold to [-pi, pi)
            ge1 = wp.tile([P, NBLK * 8], F32, tag="ge1")
            ge2 = wp.tile([P, NBLK * 8], F32, tag="ge2")
            ge3 = wp.tile([P, NBLK * 8], F32, tag="ge3")
            nc.vector.tensor_scalar(out=ge1[:], in0=arg[:], scalar1=float(2 * pi),
                                    scalar2=float(2 * pi), op0=OP.is_ge, op1=OP.mult)
            nc.gpsimd.tensor_scalar(out=ge2[:], in0=arg[:], scalar1=float(4 * pi),
                                    scalar2=float(2 * pi), op0=OP.is_ge, op1=OP.mult)
            nc.vector.tensor_scalar(out=ge3[:], in0=arg[:], scalar1=float(6 * pi),
                                    scalar2=float(2 * pi), op0=OP.is_ge, op1=OP.mult)
            nc.gpsimd.tensor_tensor(out=ge2[:], in0=ge2[:], in1=ge3[:], op=OP.add)
            nc.vector.tensor_tensor(out=arg[:], in0=arg[:], in1=ge1[:], op=OP.subtract)
            nc.vector.tensor_tensor(out=arg[:], in0=arg[:], in1=ge2[:], op=OP.subtract)
            nc.vector.tensor_scalar(out=ge1[:], in0=arg[:], scalar1=float(pi),
                                    scalar2=float(2 * pi), op0=OP.is_ge, op1=OP.mult)
            nc.vector.tensor_tensor(out=arg[:], in0=arg[:], in1=ge1[:], op=OP.subtract)
            # angular monomials ang [128, (blk, i)] scaled by sqrt(PREF), on DVE
            # while gpsimd reduces the bessel arguments
            ang = pp.tile([P, NBLK * N_L], F32)
            angv = ang[:].rearrange("p (b i) -> p b i", i=N_L)
            nc.vector.tensor_scalar(out=angv[:, :, 0], in0=uu[:], scalar1=0.0, scalar2=1.0,
                                    op0=OP.mult, op1=OP.add)
            nc.vector.tensor_copy(
                angv[:, :, 1:4],
                U[:].rearrange("p (c b) -> p b c", c=3))
            for lo, hi, plo, c in _CHAIN_BATCH:
                cnt = hi - lo
                nc.vector.tensor_tensor(
                    out=angv[:, :, lo:hi],
                    in0=angv[:, :, plo:plo + cnt],
                    in1=U[:, c * NBLK:(c + 1) * NBLK][:, :, None].to_broadcast([P, NBLK, cnt]),
                    op=OP.mult)
            # fold the sqrt(PREF) prefactor into the bf16 conversion
            ang16 = pp.tile([P, NBLK * N_L], BF16)
            ang16v = ang16[:].rearrange("p (b i) -> p b i", i=N_L)
            nc.vector.tensor_tensor(
                out=ang16v,
                in0=angv[:],
                in1=sprow[:, None, :].to_broadcast([P, NBLK, N_L]),
                op=OP.mult)

            sinv = wp.tile([P, NBLK * 8], F32, tag="sinv")
            nc.scalar.activation(sinv[:], arg[:], AF.Sin)
            WT16 = pp.tile([P, 180], BF16)
            nc.scalar.copy(WT16[:], wpack[:, 32:212])
            ident16 = pp.tile([P, P], BF16)
            nc.scalar.copy(ident16[:], blkdiag[:])
            # cutoff polynomial
            u2 = wp.tile([P, NBLK], F32, tag="u2")
            nc.vector.tensor_tensor(out=u2[:], in0=uu[:], in1=uu[:], op=OP.mult)
            a1 = wp.tile([P, NBLK], F32, tag="a1")
            nc.vector.tensor_scalar(out=a1[:], in0=uu[:], scalar1=-48.0, scalar2=28.0,
                                    op0=OP.mult, op1=OP.add)
            g21 = wp.tile([P, NBLK], F32, tag="g21")
            nc.vector.tensor_scalar_mul(g21[:], u2[:], 21.0)
            nc.vector.tensor_tensor(out=g21[:], in0=g21[:], in1=a1[:], op=OP.add)
            u6 = wp.tile([P, NBLK], F32, tag="u6")
            nc.vector.tensor_tensor(out=u6[:], in0=u2[:], in1=u2[:], op=OP.mult)
            nc.vector.tensor_tensor(out=u6[:], in0=u6[:], in1=u2[:], op=OP.mult)
            fc = wp.tile([P, NBLK], F32, tag="fc")
            nc.vector.tensor_tensor(out=fc[:], in0=u6[:], in1=g21[:], op=OP.mult)
            nc.vector.tensor_scalar(out=fc[:], in0=fc[:], scalar1=-1.0, scalar2=1.0,
                                    op0=OP.mult, op1=OP.add)
            lt = wp.tile([P, NBLK], F32, tag="lt")
            nc.vector.tensor_scalar(out=lt[:], in0=uu[:], scalar1=1.0, scalar2=None, op0=OP.is_lt)
            nc.vector.tensor_tensor(out=fc[:], in0=fc[:], in1=lt[:], op=OP.mult)
            scal = wp.tile([P, NBLK], F32, tag="scal")
            nc.vector.tensor_tensor(out=scal[:], in0=rinv[:], in1=fc[:], op=OP.mult)
            nc.vector.tensor_scalar_mul(scal[:], scal[:], float(np.sqrt(2.0 / CUTOFF)))
            # rc in bf16 (single rounding from the f32 product)
            rc16 = pp.tile([P, NBLK * 8], BF16)
            nc.vector.tensor_tensor(
                out=rc16[:].rearrange("p (b r) -> p b r", r=8),
                in0=sinv[:].rearrange("p (b r) -> p b r", r=8),
                in1=scal[:, :, None].to_broadcast([P, NBLK, 8]), op=OP.mult)

            # G1 [128, (blk, i, a)] bf16, per 12-block quarter (all-bf16 2x)
            G1 = pp.tile([P, NBLK * N_L * NAB], BF16)

            def build_g1(g8):
                bs = slice(g8 * 12, g8 * 12 + 12)
                nc.vector.tensor_tensor(
                    out=G1[:].rearrange("p (b i a) -> p b i a", i=N_L, a=NAB)[:, bs],
                    in0=ang16v[:, bs, :, None].to_broadcast([P, 12, N_L, NAB]),
                    in1=embsE16.rearrange("p (b a) -> p b a", a=NAB)[:, bs, None, :].to_broadcast([P, 12, N_L, NAB]),
                    op=OP.mult)

            # sw1 [128, (blk, r, n)] bf16: rc x recv one-hot, per 12-block quarter
            sw1 = pp.tile([P, NBLK * P], BF16)

            def build_sw1(g8):
                bs = slice(g8 * 12, g8 * 12 + 12)
                nc.vector.tensor_tensor(
                    out=sw1[:].rearrange("p (b r n) -> p b r n", r=8, n=16)[:, bs],
                    in0=recv16.rearrange("p (b n) -> p b n", n=SUBN)[:, bs, None, :].to_broadcast([P, 12, 8, 16]),
                    in1=rc16[:].rearrange("p (b r) -> p b r", r=8)[:, bs, :, None].to_broadcast([P, 12, 8, 16]),
                    op=OP.mult)

            # parity-split S_w in fp8 for stage-2 DoubleRow msg_A. The zero
            # half is DMA-broadcast-filled (off-engine); only the 6144
            # nonzeros (at r = 2*rp + q, a linear-stride AP) are computed.
            sw2 = pp.tile([P, NBLK * 256], FP8)
            nc.sync.dma_start(out=sw2[:].bitcast(BF16), in_=x_zero[:])
            sw2nz = sw2[:].rearrange("p (b q rp par n) -> p b q rp par n",
                                     q=2, rp=4, par=2, n=16)
            rc16r = rc16[:].rearrange("p (b rp par) -> p b rp par", rp=4, par=2)

            def build_sw2(q, half, eng):
                bs = slice(half * 24, half * 24 + 24)
                eng.tensor_tensor(
                    out=sw2nz[:, bs, q, :, q, :],
                    in0=recv16.rearrange("p (b n) -> p b n", n=SUBN)[:, bs, None, :].to_broadcast([P, 24, 4, 16]),
                    in1=rc16r[:, bs, :, q][:, :, :, None].to_broadcast([P, 24, 4, 16]),
                    op=OP.mult)

            def symmetrize_pool(bv, sv, ns):
                # bv [P,ns,5,c]; sv [P,ns,20,c]: sum-of-squares tree on gpsimd
                # (bv[:,:,0,:] filled by the caller via Act copy)
                s5 = wp.tile([P, 8 * 5 * CHAN], F32, tag="ps5")
                v5 = s5[:].rearrange("p (s i c) -> p s i c", i=5, c=CHAN)[:, 0:ns]
                s3 = wp.tile([P, 8 * 3 * CHAN], F32, tag="ps3")
                v3 = s3[:].rearrange("p (s i c) -> p s i c", i=3, c=CHAN)[:, 0:ns]
                TT = nc.gpsimd.tensor_tensor
                nc.gpsimd.tensor_copy(bv[:, :, 1, :], sv[:, :, 0, :])
                # l=1: i 1..4
                TT(out=v3[:, :, 0, :], in0=sv[:, :, 1, :], in1=sv[:, :, 2, :], op=OP.add)
                TT(out=bv[:, :, 2, :], in0=v3[:, :, 0, :], in1=sv[:, :, 3, :], op=OP.add)
                # l=2: i 4..10
                TT(out=v3[:], in0=sv[:, :, 4:7, :], in1=sv[:, :, 7:10, :], op=OP.add)
                TT(out=v5[:, :, 0, :], in0=v3[:, :, 0, :], in1=v3[:, :, 1, :], op=OP.add)
                TT(out=bv[:, :, 3, :], in0=v5[:, :, 0, :], in1=v3[:, :, 2, :], op=OP.add)
                # l=3: i 10..20
                TT(out=v5[:], in0=sv[:, :, 10:15, :], in1=sv[:, :, 15:20, :], op=OP.add)
                TT(out=v3[:, :, 0:2, :], in0=v5[:, :, 0:2, :], in1=v5[:, :, 2:4, :], op=OP.add)
                TT(out=v3[:, :, 2, :], in0=v3[:, :, 0, :], in1=v3[:, :, 1, :], op=OP.add)
                TT(out=bv[:, :, 4, :], in0=v3[:, :, 2, :], in1=v5[:, :, 4, :], op=OP.add)

            A16 = pp.tile([P, NSUB * 180], BF16)
            A8 = pp.tile([P, NSUB * 180], FP8)
            mem16 = pp.tile([P, NSUB * 180], BF16)
            B0_all = pp.tile([P, NSUB * 45], F32)
            B1_all = pp.tile([P, NSUB * 45], F32)
            red1 = pp.tile([P, NSUB * CHAN], F32)
            chic = pp.tile([16, NSUB * CHAN], F32)
            Vsb = pp.tile([16, NSUB * CHAN], BF16)

            tabsh = dr.tile([NROW, TABW], FP8)
            tabfull = dr.tile([NC * NROW, TABW], FP8)

            # ---------- stage 1: seg-sum + RT for all 4 groups first (PE/Act
            # critical path unblocked), node-level phases stream behind ----------
            s1ctx = tc.tile_pool(name="ps_s1", bufs=2, space="PSUM")
            ps_s1 = s1ctx.__enter__()
            t1gs = []
            if PH & 2:
                build_g1(0)
                build_sw1(0)
            for g8 in range(4 if (PH & 2) else 0):
                if g8 < 3:
                    build_g1(g8 + 1)
                    build_sw1(g8 + 1)
                t1g = ps_s1.tile([P, 480], F32, space="PSUM", tag="t1g", bufs=4)
                t1gs.append(t1g)
                pend = []
                for j in range(9):
                    if j < 8:
                        s = g8 * 8 + j
                        t0 = ps_s1.tile([P, 60], F32, space="PSUM", tag="t0", bufs=3)
                        ranges = _block_ranges(s)
                        for mi, (blk, p0, p1) in enumerate(ranges):
                            nc.tensor.matmul(t0[:], lhsT=sw1[p0:p1, blk * 128:(blk + 1) * 128],
                                             rhs=G1[p0:p1, blk * 60:(blk + 1) * 60],
                                             start=(mi == 0), stop=(mi == len(ranges) - 1))
                        t0c = wp.tile([P, 60], BF16, tag="t0c", bufs=4)
                        nc.scalar.copy(t0c[:], t0[:])
                        pend.append((j, t0c))
                    if (j > 0 or g8 > 0) and pend and (j == 8 or len(pend) > 1):
                        jj, t0cp = pend.pop(0)
                        for l, (a, b) in enumerate(L_RANGES):
                            nc.tensor.matmul(
                                t1g[:, jj * 60 + a * NAB: jj * 60 + b * NAB],
                                lhsT=rtl16[l][:], rhs=t0cp[:, a * NAB:b * NAB],
                                start=True, stop=True)
            for g8 in range(4 if (PH & 4) else 0):
                # ---- group node-level: A16, A8, B0, chi, V, repack, AG slice ----
                t1g = t1gs[g8]
                sl = slice(g8 * 1440, (g8 + 1) * 1440)
                sl45 = slice(g8 * 360, (g8 + 1) * 360)
                sl9 = slice(g8 * 72, (g8 + 1) * 72)
                # A16 holds the unscaled A (bf16); MP_NORM enters only in the
                # fp8 A8 copy (table) and chic
                nc.vector.tensor_tensor(
                    out=A16[:, sl].rearrange("p (j ia b) -> p j ia b", j=8, b=NAB),
                    in0=t1g[:].rearrange("p (j ia) -> p j ia", j=8)[:, :, :, None].to_broadcast([P, 8, 60, NAB]),
                    in1=EM[:, g8 * 24:(g8 + 1) * 24].rearrange("p (j b) -> p j b", b=NAB)[:, :, None, :].to_broadcast([P, 8, 60, NAB]),
                    op=OP.mult)
                nc.scalar.activation(A8[:, sl], A16[:, sl], AF.Copy, scale=float(MP_NORM))
                scr = wp.tile([P, 1440], F32, tag="scr")
                nc.vector.tensor_tensor(out=scr[:], in0=A16[:, sl], in1=A16[:, sl], op=OP.mult)
                bv = B0_all[:, sl45].rearrange("p (s l c) -> p s l c", l=5, c=CHAN)
                sv = scr[:].rearrange("p (s i c) -> p s i c", i=N_L, c=CHAN)
                nc.gpsimd.tensor_copy(
                    bv[:, :, 0, :],
                    A16[:, sl].rearrange("p (s i c) -> p s i c", i=N_L, c=CHAN)[:, :, 0, :])
                rv = red1[:, sl9].rearrange("p (s c) -> p s c", c=CHAN)
                if g8 == 3:
                    # last group: chi computed straight from the squares so the
                    # final AG slice doesn't wait on the symmetrize tree
                    nc.vector.tensor_reduce(
                        out=rv, in_=sv.transpose([0, 1, 3, 2]),
                        axis=mybir.AxisListType.X, op=OP.add)
                    nc.vector.tensor_tensor(out=rv, in0=rv, in1=bv[:, :, 0, :], op=OP.add)
                    symmetrize_pool(bv, sv, 8)
                else:
                    symmetrize_pool(bv, sv, 8)
                    nc.vector.tensor_reduce(
                        out=rv, in_=bv.transpose([0, 1, 3, 2]),
                        axis=mybir.AxisListType.X, op=OP.add)
                chips = ps_s1.tile([16, 72], F32, space="PSUM", tag="chips", bufs=1)
                nc.tensor.matmul(chips[:], lhsT=blkdiag[:, 0:16], rhs=red1[:, sl9],
                                 start=True, stop=True)
                nc.vector.tensor_scalar_mul(chic[:, sl9], chips[:], float(MP_NORM))
                nc.vector.tensor_tensor(
                    out=Vsb[:, sl9].rearrange("p (s a b) -> p s a b", a=NAB, b=NAB),
                    in0=chic[:, sl9].rearrange("p (s a b) -> p s a b", a=NAB, b=NAB),
                    in1=EM[0:16, g8 * 24:(g8 + 1) * 24].rearrange("p (s a) -> p s a", a=NAB)[:, :, :, None].to_broadcast([16, 8, NAB, NAB]),
                    op=OP.mult)
                if (PH & 8) and g8 % 2 == 1:
                    # repack super-group: A rows (fp8) + V column (bf16 in the
                    # row pad) for 16 subtiles (256 table rows); then AG slice
                    sg = g8 // 2
                    ssl = slice(sg * 2880, (sg + 1) * 2880)
                    ssl9 = slice(sg * 144, (sg + 1) * 144)
                    for sp in range(8):
                        nc.sync.dma_start(
                            out=tabsh[:].rearrange("(s n) w -> n s w", n=SUBN)[:, sg * 16:(sg + 1) * 16, sp * 180:(sp + 1) * 180],
                            in_=A8[sp * 16:(sp + 1) * 16, ssl].rearrange("n (s f) -> n s f", f=180))
                    nc.sync.dma_start(
                        out=tabsh[:].rearrange("(s n) w -> n s w", n=SUBN)[:, sg * 16:(sg + 1) * 16, 1440:1458].bitcast(BF16),
                        in_=Vsb[:, ssl9].rearrange("n (s c) -> n s c", c=CHAN))
                    rsl = slice(sg * 256, (sg + 1) * 256)
                    if sim_mode:
                        # stand-in for the sliced AllGather: 4 local copies per
                        # slice model the 8-core AG of the 0.77MB/rank fp8
                        # shard (same total bytes as the real collective)
                        for _cc in range(4):
                            nc.sync.dma_start(
                                tabfull[sg * NC * 256 + _cc * 256:
                                        sg * NC * 256 + (_cc + 1) * 256, :],
                                tabsh[rsl, :])
                    else:
                        # tabfull rows are (slice, core, 256): slice output is
                        # the contiguous rank-major block for this slice
                        nc.gpsimd.collective_compute(
                            "AllGather", mybir.AluOpType.bypass,
                            replica_groups=[list(range(NC))],
                            ins=[tabsh[rsl, :]],
                            outs=[tabfull[sg * NC * 256:(sg + 1) * NC * 256, :]])
            s1ctx.__exit__(None, None, None)

            # sw2 fp8 build: fills the AllGather window (wait hint keeps it
            # off the stage-1 critical DVE stream)
            if PH & 16:
                with tc.tile_wait_until(0.034):
                    build_sw2(0, 0, nc.vector)
                    build_sw2(1, 0, nc.gpsimd)
                    build_sw2(0, 1, nc.vector)
                    build_sw2(1, 1, nc.vector)
            sw2v = sw2[:].rearrange("p (b q rn) -> p b q rn", q=2, rn=128)
            # memory term: scheduled into the AllGather/stage-2 DVE idle time
            with tc.tile_wait_until(0.036):
                nc.vector.tensor_tensor(
                    out=mem16[:].rearrange("p (s f) -> p s f", f=180),
                    in0=A16[:].rearrange("p (s f) -> p s f", f=180),
                    in1=WT16[:, None, :].to_broadcast([P, NSUB, 180]),
                    op=OP.mult)

            # ---------- stage 2 (1-deep software pipeline: pair gg's
            # gather-independent matmuls are emitted before pair gg-1's
            # t2s-dependent tail, so PE never stalls on the DVE hop) ----------
            s2ctx = tc.tile_pool(name="ps_s2", bufs=2, space="PSUM")
            ps_s2 = s2ctx.__enter__()
            gat4 = None
            NP = NSUB // 2 if (PH & 16) else 0
            hist = {}
            for gg in range(NP + 1 if NP else 0):
                if gg < NP:
                    if gg % 2 == 0:
                        g4 = gg // 2
                        gat4 = wp.tile([P, 6, TABW], FP8, tag="gat", bufs=3)
                        nc.gpsimd.dma_gather(gat4[:], tabfull[:],
                                             gidx[:, g4 * 48:(g4 + 1) * 48],
                                             GB * CAP, GB * CAP, TABW)
                    b3 = (gg % 2) * 3             # this pair's blocks within gat4
                    gatv = gat4[:, b3:b3 + 3, 1440:1458].bitcast(BF16)
                    G2 = wp.tile([P, 3, 180], BF16, tag="g2", bufs=3)
                    nc.vector.tensor_tensor(
                        out=G2[:].rearrange("p b (i c) -> p b i c", c=CHAN),
                        in0=ang16v[:, 3 * gg:3 * gg + 3, :, None].to_broadcast([P, 3, N_L, CHAN]),
                        in1=gatv[:, :, None, :].to_broadcast([P, 3, N_L, CHAN]),
                        op=OP.mult)
                    t2pair = ps_s2.tile([P, 360], F32, space="PSUM", tag="t2", bufs=3)
                    a1pair = ps_s2.tile([P, 360], F32, space="PSUM", tag="a1p", bufs=3)
                    hist[gg] = (t2pair, a1pair)
                    for s2 in range(2):
                        s = gg * 2 + s2
                        osl = slice(s2 * 180, (s2 + 1) * 180)
                        ranges = _block_ranges(s)
                        for mi, (blk, p0, p1) in enumerate(ranges):
                            bloc = blk - 3 * gg + b3
                            nc.tensor.matmul(
                                t2pair[:, osl],
                                lhsT=sw1[p0:p1, blk * 128:(blk + 1) * 128],
                                rhs=G2[p0:p1, bloc - b3, :],
                                start=(mi == 0), stop=(mi == len(ranges) - 1))
                        # msg_A: fp8 DoubleRow, radial parity pair as the k-tiles
                        for ri, (blk, p0, p1) in enumerate(ranges):
                            bloc = blk - 3 * gg + b3
                            for k in range(4):
                                nc.tensor.matmul(
                                    a1pair[k * 32:(k + 1) * 32, osl],
                                    lhsT=sw2v[p0:p1, blk, :, k * 32:(k + 1) * 32],
                                    rhs=gat4[p0:p1, bloc, 2 * k * 180:(2 * k + 2) * 180].rearrange(
                                        "p (q f) -> p q f", q=2),
                                    start=(ri == 0), stop=False,
                                    perf_mode=DR, tile_position=(p0, k * 32))
                if gg == 0:
                    continue
                gp = gg - 1
                t2pair, a1pair = hist.pop(gp)
                t2s = wp.tile([P, 360], BF16, tag="t2s", bufs=3)
                for s2 in range(2):
                    s = gp * 2 + s2
                    osl = slice(s2 * 180, (s2 + 1) * 180)
                    nc.vector.tensor_tensor(
                        out=t2s[:, osl].rearrange("p (i a b) -> p i a b", a=NAB, b=NAB),
                        in0=t2pair[:, osl].rearrange("p (i a b) -> p i a b", a=NAB, b=NAB),
                        in1=EM[:, s * NAB:(s + 1) * NAB][:, None, None, :].to_broadcast([P, N_L, NAB, NAB]),
                        op=OP.mult)
                    for l, (a, b) in enumerate(L_RANGES):
                        nc.tensor.matmul(
                            a1pair[:, s2 * 180 + a * CHAN: s2 * 180 + b * CHAN],
                            lhsT=rtl16[l][:], rhs=t2s[:, s2 * 180 + a * CHAN: s2 * 180 + b * CHAN],
                            start=False, stop=False)
                    # fold the memory term into the PSUM group (identity matmul)
                    nc.tensor.matmul(
                        a1pair[:, osl], lhsT=ident16[:],
                        rhs=mem16[:, s * 180:(s + 1) * 180],
                        start=False, stop=True)
                if not (PH & 32):
                    continue
                # ---- stage 2 node-level per pair: B1 + output ----
                sl45 = slice(gp * 90, (gp + 1) * 90)
                bv = B1_all[:, sl45].rearrange("p (s l c) -> p s l c", l=5, c=CHAN)
                a1v = a1pair[:].rearrange("p (s i c) -> p s i c", i=N_L, c=CHAN)
                scr1 = wp.tile([P, 360], F32, tag="scr1", bufs=3)
                nc.scalar.activation(scr1[:], a1pair[:], AF.Square)
                nc.scalar.copy(bv[:, :, 0, :], a1v[:, :, 0, :])
                sv = scr1[:].rearrange("p (s i c) -> p s i c", i=N_L, c=CHAN)
                for l, (a, b) in enumerate(L_RANGES):
                    nc.vector.tensor_reduce(
                        out=bv[:, :, l + 1, :],
                        in_=sv[:, :, a:b, :].transpose([0, 1, 3, 2]),
                        axis=mybir.AxisListType.X, op=OP.add)
                nc.sync.dma_start(o_b1[:, sl45], B1_all[:, sl45])
            s2ctx.__exit__(None, None, None)
            with tc.tile_wait_until(0.065):
                nc.sync.dma_start(o_b0[:], B0_all[:])
            if not (PH & 4):
                nc.sync.dma_start(o_b0[:, 0:156], cons[:])
            if not (PH & 32):
                nc.sync.dma_start(o_b1[:, 0:156], cons[:])
    nc.compile()
    return nc


# ================= public entry =================
def kernel(positions, shifts, W_emb, W_rt, W_nm, atomic_numbers, edge_index):
    global _PROGRAM
    prep = _prep(positions, shifts, atomic_numbers, edge_index)
    consts = _consts()
    if _PROGRAM is None:
        _PROGRAM = _build()
    nc = _PROGRAM
    wemb = np.asarray(W_emb, dtype=np.float32)
    wrt = np.asarray(W_rt, dtype=np.float32)
    wnm = np.asarray(W_nm, dtype=np.float32)
    # host-replicated weight patterns (pure tiling/gathers of the small weights)
    pg = np.arange(P) // 16                                   # r|s' group per partition
    rtlw = wrt[:, pg, :].transpose(1, 0, 2).reshape(P, 32)    # [p, (l, s')] = W_rt[l, p//16, s']
    wtp = wnm[0, pg][:, L_OF, :].reshape(P, 180)
    parc = _parc16()
    in_maps = []
    for c in range(NC):
        em = wemb[prep["rowsp"][c].reshape(NSUB, SUBN)]       # [sub, n, a]
        em = em[:, np.arange(P) % 16, :].transpose(1, 0, 2).reshape(P, NSUB * NAB)
        wpack = np.concatenate([rtlw, wtp, em], axis=1).astype(np.float32)
        embse = wemb[prep["sendsp"][c]].reshape(P, NBLK * NAB).astype(np.float32)
        main = np.ascontiguousarray(np.concatenate(
            [prep["geo"][c], consts, wpack],
            axis=1).astype(np.float32))
        b16 = np.ascontiguousarray(np.concatenate(
            [prep["recv"][c], embse, parc, np.zeros((P, 16), np.float32)],
            axis=1).astype(ml_dtypes.bfloat16))
        in_maps.append(dict(x_main=main, x_b16=b16, x_gidx=prep["gidx"][c],
                            x_zero=np.zeros((P, NBLK * 128), ml_dtypes.bfloat16)))
    res = run_bass_kernel_spmd(nc, in_maps, list(range(NC))).results
    # unshard: [128=(s',n), (sub, l, c)] -> node rows
    out = np.zeros((N_NODES, N_RB, 5, CHAN, 2), dtype=np.float32)
    node_of_row = prep["node_of_row"]
    for c in range(NC):
        for mp, name in ((0, "o_b0"), (1, "o_b1")):
            arr = np.asarray(res[c][name], dtype=np.float32).reshape(8, SUBN, NSUB, 5, CHAN)
            rows = arr.transpose(2, 1, 0, 3, 4).reshape(NROW, N_RB, 5, CHAN)
            valid = node_of_row[c * NROW:(c + 1) * NROW] >= 0
            out[node_of_row[c * NROW:(c + 1) * NROW][valid], :, :, :, mp] = rows[valid]
    return out
